# revision 1
# baseline (speedup 1.0000x reference)
"""Trainium2 Bass kernel for a T5-style decoder layer (self-attn with causal
rel-pos bias, cross-attn, FFN, 3 post-LNs).

Sharding: 8 cores = (batch b in 0..3) x (parity g in 0..1). Core (b, g) owns
query blocks {g, 2+g, 4+g, 6+g} (128 rows each) of batch b and computes the
full layer for those 512 rows. K/V work is duplicated across the pair; no
cross-core collectives are needed. Causal score work is padded to a uniform
(2,4,6,8) key-block pattern so one SPMD program serves all cores; the padded
blocks are killed by the bias band (-inf-ish mask baked into the band).

All activations are kept in TRANSPOSED layout [d_model on partitions, tokens]
so every matmul consumes weights in their natural [d_in, d_out] layout and no
on-device transposes are needed except V (per-head) and the final output.

The rel-pos bias + causal mask enter as a per-(head) Toeplitz band tile
BC[h, i, w] = 8*band_h((w - 128 + 128*g) - i); the slice BC[:, w0:w0+128] with
w0 = 256*s - 128*kb + 128 is exactly the bias patch for (slot s, key block
kb). Host precomputes BC (g baked in), already scaled by 8 so the single
ACT Exp(scale=1/8) applies both the 1/sqrt(dk) and the bias.
"""

import functools
import math

import numpy as np

import concourse.bass as bass
import concourse.bacc as bacc
import concourse.mybir as mybir
import concourse.tile as tile
from concourse.bass_utils import run_bass_kernel_spmd
from concourse.masks import make_identity

F32 = mybir.dt.float32
F32R = mybir.dt.float32r
AL = mybir.AluOpType
AF = mybir.ActivationFunctionType

B, L, D, H, DK, DFF = 4, 1024, 1024, 16, 64, 4096
P = 128
NB = D // P            # 8 d_model blocks
NF = DFF // P          # 32 d_ff blocks
TOK = 512              # tokens owned per core
SLOTS = 4              # query blocks of 128 per core
NUM_BUCKETS, MAX_DISTANCE = 32, 128
EPS = 1e-5
MASK8 = -480.0         # 8 * (-60); exp((S-480)/8) == exp(S/8 - 60) ~ 0


def _r(x):
    return x.bitcast(F32R)


def _f(x):
    return x.bitcast(F32)


def _build_nc(reps=1):
    nc = bacc.Bacc(trn_type="TRN2")

    def inp(name, shape, dt=F32):
        return nc.declare_dram_parameter(name, list(shape), dt, isOutput=False)

    d_xoT = inp("x_ownT", (D, TOK), F32R)
    d_xfT = inp("x_fullT", (D, L), F32R)
    d_memT = inp("memT", (D, L), F32R)
    d_bc = inp("bc", (H, P, 1280), F32R)
    dw = {}
    for pre in ("sa", "ca"):
        for nm in ("wq", "wk", "wv", "wo"):
            dw[f"{pre}_{nm}"] = inp(f"{pre}_{nm}", (NB, P, NB, P), F32R)
        for nm in ("bq", "bk", "bv", "bo"):
            dw[f"{pre}_{nm}"] = inp(f"{pre}_{nm}", (P, NB))
    d_fc1 = inp("fc1_w", (NF, P, NB, P), F32R)
    d_fc2 = inp("fc2_w", (NB, P, NF, P), F32R)
    d_fc1b = inp("fc1_b", (P, NF))
    d_fc2b = inp("fc2_b", (P, NB))
    dln = {}
    for i in ("1", "2", "3"):
        dln[f"g{i}"] = inp(f"ln{i}_g", (P, NB))
        dln[f"b{i}"] = inp(f"ln{i}_b", (P, NB))
    d_out = nc.declare_dram_parameter("out_own", [TOK, D], F32, isOutput=True)


    with (
        nc.allow_low_precision(reason="f32r matmul inputs are rounded"),
        tile.TileContext(nc) as tc,
    ):
        with tc.tile_pool(name="persist", bufs=1) as pers:
            identf = pers.tile([P, P], F32, tag="identf")
            make_identity(nc, identf[:])
            ident = pers.tile([P, P], F32R, tag="ident")
            nc.scalar.copy(out=ident[:], in_=identf[:])
            onesf = pers.tile([P, P], F32, tag="onesf")
            nc.gpsimd.memset(onesf[:], 1.0)
            ones_col = pers.tile([P, 1], F32R, tag="ones")
            nc.scalar.copy(out=ones_col[:], in_=onesf[:, 0:1])
            ones_row = pers.tile([1, P], F32R, tag="onesr")
            nc.scalar.copy(out=ones_row[:], in_=onesf[0:1, :])
            ones_nb1 = pers.tile([P, NB, 1], F32, tag="onesnb")
            nc.gpsimd.memset(ones_nb1[:], 1.0)
            eps_t = pers.tile([1, 1], F32, tag="epsc")
            nc.gpsimd.memset(eps_t[:], EPS)

            for _rep in range(reps):
                bias_sb = {}
                for k in dw:
                    if k[3] == "b":
                        t = pers.tile([P, NB], F32, tag=f"b_{k}", name=f"b_{k}")
                        nc.sync.dma_start(out=t[:], in_=dw[k][:, :])
                        bias_sb[k] = t
                fc1b = pers.tile([P, NF], F32, tag="fc1b")
                nc.sync.dma_start(out=fc1b[:], in_=d_fc1b[:, :])
                fc2b = pers.tile([P, NB], F32, tag="fc2b")
                nc.sync.dma_start(out=fc2b[:], in_=d_fc2b[:, :])
                ln_sb = {}
                for k, dv in dln.items():
                    t = pers.tile([P, NB], F32, tag=f"ln_{k}", name=f"ln_{k}")
                    nc.sync.dma_start(out=t[:], in_=dv[:, :])
                    ln_sb[k] = t

                def layernorm_T(src, g_ap, b_ap, out_tiles, pool, psum):
                    pm = psum.tile([1, TOK], F32, tag="pgen", name="pm", bufs=2)
                    for d in range(NB):
                        nc.tensor.matmul(pm[:], _r(ones_col[:]), _r(src[d][:]),
                                         start=(d == 0), stop=(d == NB - 1))
                    pv2 = psum.tile([1, TOK], F32, tag="pgen", name="pv2", bufs=2)
                    for d in range(NB):
                        sq = pool.tile([P, TOK], F32R, tag="sq", bufs=2)
                        nc.scalar.square(sq[:], _f(src[d][:]))
                        nc.tensor.matmul(pv2[:], _r(ones_col[:]), _r(sq[:]),
                                         start=(d == 0), stop=(d == NB - 1))
                    mu = pool.tile([1, TOK], F32, tag="mu")
                    nc.vector.tensor_scalar_mul(mu[:], pm[:], 1.0 / D)
                    musq = pool.tile([1, TOK], F32, tag="musq")
                    nc.vector.tensor_mul(musq[:], mu[:], mu[:])
                    var = pool.tile([1, TOK], F32, tag="var")
                    nc.vector.scalar_tensor_tensor(
                        var[:], pv2[:], 1.0 / D, musq[:],
                        op0=AL.mult, op1=AL.subtract)
                    std = pool.tile([1, TOK], F32, tag="std")
                    nc.scalar.activation(std[:], var[:], AF.Sqrt, bias=eps_t[:])
                    rsd = pool.tile([1, TOK], F32R, tag="rsd")
                    nc.vector.reciprocal(rsd[:], std[:])
                    nmr = pool.tile([1, TOK], F32R, tag="nmr")
                    nc.vector.tensor_mul(nmr[:], mu[:], _f(rsd[:]))
                    nc.vector.tensor_scalar_mul(nmr[:], nmr[:], -1.0)
                    rsdr = pool.tile([P, TOK], F32, tag="rsdr")
                    nmrr = pool.tile([P, TOK], F32, tag="nmrr")
                    for row, rep in ((rsd, rsdr), (nmr, nmrr)):
                        pb = psum.tile([P, TOK], F32, tag="pgen", name="pbc",
                                       bufs=2)
                        nc.tensor.matmul(pb[:], _r(ones_row[:]), _r(row[:]))
                        nc.scalar.copy(out=rep[:], in_=pb[:])
                    for d in range(NB):
                        g_sl = g_ap[:, d:d + 1]
                        t1 = pool.tile([P, TOK], F32, tag="t1", bufs=2)
                        nc.vector.scalar_tensor_tensor(
                            t1[:], _f(src[d][:]), g_sl, rsdr[:],
                            op0=AL.mult, op1=AL.mult)
                        t2 = pool.tile([P, TOK], F32, tag="t2", bufs=2)
                        nc.vector.scalar_tensor_tensor(
                            t2[:], nmrr[:], g_sl, t1[:],
                            op0=AL.mult, op1=AL.add)
                        nc.scalar.activation(out_tiles[d][:], t2[:], AF.Identity,
                                             bias=b_ap[:, d:d + 1], scale=1.0)

                def attention(q_src, kvT, pre, causal, out_tiles, resid,
                              pool, psum):
                    wqv, wkv = dw[f"{pre}_wq"], dw[f"{pre}_wk"]
                    wvv, wov = dw[f"{pre}_wv"], dw[f"{pre}_wo"]
                    bq, bk = bias_sb[f"{pre}_bq"], bias_sb[f"{pre}_bk"]
                    bv, bo = bias_sb[f"{pre}_bv"], bias_sb[f"{pre}_bo"]
                    AO = [pool.tile([P, TOK], F32R, tag=f"ao{hp}", name=f"ao{hp}",
                                    bufs=1) for hp in range(NB)]
                    for hp in range(NB):
                        wq_t = pool.tile([P, NB, P], F32R, tag="wqt", bufs=2)
                        nc.sync.dma_start(out=wq_t[:],
                                          in_=wqv[hp])
                        wk_t = pool.tile([P, NB, P], F32R, tag="wkt", bufs=2)
                        nc.sync.dma_start(out=wk_t[:],
                                          in_=wkv[hp])
                        wv_t = pool.tile([P, NB, P], F32R, tag="wvt", bufs=1)
                        nc.sync.dma_start(out=wv_t[:],
                                          in_=wvv[hp])

                        pq = psum.tile([P, TOK], F32, tag="pgen", name="pq", bufs=2)
                        for t in range(NB):
                            nc.tensor.matmul(pq[:], _r(wq_t[:, t, :]),
                                             _r(q_src[t][:]),
                                             start=(t == 0), stop=(t == NB - 1))
                        q_sb = pool.tile([P, TOK], F32R, tag="qsb", bufs=2)
                        nc.vector.tensor_scalar_add(q_sb[:], pq[:],
                                                    bq[:, hp:hp + 1])

                        pk = psum.tile([P, L], F32, tag="pkv", name="pk", bufs=1)
                        for half in range(2):
                            sl = slice(half * 512, (half + 1) * 512)
                            for t in range(NB):
                                nc.tensor.matmul(pk[:, sl], _r(wk_t[:, t, :]),
                                                 _r(kvT[t][:, sl]),
                                                 start=(t == 0),
                                                 stop=(t == NB - 1))
                        k_sb = pool.tile([P, L], F32R, tag="ksb", bufs=1)
                        nc.vector.tensor_scalar_add(k_sb[:], pk[:],
                                                    bk[:, hp:hp + 1])

                        pv = psum.tile([P, L], F32, tag="pkv", name="pv", bufs=1)
                        for half in range(2):
                            sl = slice(half * 512, (half + 1) * 512)
                            for t in range(NB):
                                nc.tensor.matmul(pv[:, sl], _r(wv_t[:, t, :]),
                                                 _r(kvT[t][:, sl]),
                                                 start=(t == 0),
                                                 stop=(t == NB - 1))
                        vT_sb = pool.tile([P, L], F32R, tag="vsb", bufs=1)
                        nc.scalar.activation(vT_sb[:], pv[:], AF.Identity,
                                             bias=bv[:, hp:hp + 1], scale=1.0)

                        vn = []
                        for hh in range(2):
                            vh = pool.tile([P, NB, 65], F32R, tag=f"vn{hh}",
                                           name=f"vn{hh}", bufs=2)
                            nc.scalar.copy(out=vh[:, :, 64:65], in_=ones_nb1[:])
                            for tb in range(NB):
                                pvt = psum.tile([P, 64], F32, tag="pva",
                                                name="pvt", bufs=2)
                                nc.tensor.matmul(
                                    _r(pvt[:]),
                                    _r(vT_sb[hh * 64:hh * 64 + 64,
                                             tb * P:(tb + 1) * P]),
                                    _r(ident[hh * 64:hh * 64 + 64,
                                             hh * 64:hh * 64 + 64]),
                                    is_transpose=True)
                                nc.vector.tensor_copy(vh[:, tb, 0:64],
                                                  _f(pvt[:]))
                            vn.append(vh)

                        for hh in range(2):
                            h = 2 * hp + hh
                            hsl = slice(hh * 64, hh * 64 + 64)
                            if causal:
                                bc_t = pool.tile([P, 1280], F32R, tag="bct",
                                                 bufs=1)
                                nc.sync.dma_start(out=bc_t[:], in_=d_bc[h])
                            pts = []
                            for kb in range(8):
                                smin = kb // 2 if causal else 0
                                n = TOK - smin * P
                                ps = psum.tile([P, TOK], F32, tag="ps", name="ps",
                                               bufs=2)
                                if causal:
                                    ns = SLOTS - smin
                                    w0 = 256 * smin - 128 * kb + 128
                                    bview = bc_t[:, w0:w0 + ns * 256].rearrange(
                                        "p (s c) -> p s c", c=256)[:, :, 0:P]
                                    nc.tensor.matmul(ps[:, 0:n], _r(ident[:]),
                                                     _r(bview),
                                                     start=True, stop=False)
                                    nc.tensor.matmul(
                                        ps[:, 0:n],
                                        _r(k_sb[hsl, kb * P:(kb + 1) * P]),
                                        _r(q_sb[hsl, smin * P:TOK]),
                                        start=False, stop=True)
                                else:
                                    nc.tensor.matmul(
                                        ps[:, 0:n],
                                        _r(k_sb[hsl, kb * P:(kb + 1) * P]),
                                        _r(q_sb[hsl, smin * P:TOK]),
                                        start=True, stop=True)
                                pt = pool.tile([P, TOK], F32R, tag="pt", bufs=8)
                                nc.scalar.activation(pt[:, 0:n], ps[:, 0:n],
                                                     AF.Exp, scale=0.125)
                                pts.append(pt)
                            rec4 = pool.tile([1, TOK], F32R, tag="rec4",
                                             bufs=2)
                            pav = psum.tile([65, TOK], F32, tag="pva",
                                            name="pav", bufs=2)
                            for kb in range(8):
                                smin = kb // 2 if causal else 0
                                n = TOK - smin * P
                                nc.tensor.matmul(
                                    pav[:, smin * P:TOK], _r(vn[hh][:, kb, :]),
                                    _r(pts[kb][:, 0:n]),
                                    start=(kb == 0), stop=(kb == 7))
                            with nc.allow_low_precision(
                                    reason="f32r recip rows"):
                                nc.vector.reciprocal(rec4[:], pav[64:65, :])
                            prr = psum.tile([64, TOK], F32, tag="ps", name="prr",
                                            bufs=2)
                            nc.tensor.matmul(prr[:], _r(ones_row[0:1, 0:64]),
                                             _r(rec4[:]))
                            rrep = pool.tile([64, TOK], F32, tag="rrep", bufs=2)
                            nc.scalar.copy(out=rrep[:], in_=prr[:])
                            nc.vector.tensor_mul(AO[hp][hsl, :], pav[0:64, :],
                                                 rrep[:])

                    for db in range(NB):
                        wo_t = pool.tile([P, NB, P], F32R, tag="wot", bufs=2)
                        nc.sync.dma_start(out=wo_t[:],
                                          in_=wov[db])
                        po = psum.tile([P, TOK], F32, tag="pgen", name="po",
                                       bufs=2)
                        for hp in range(NB):
                            nc.tensor.matmul(po[:], _r(wo_t[:, hp, :]),
                                             _r(AO[hp][:]),
                                             start=(hp == 0), stop=(hp == NB - 1))
                        nc.vector.scalar_tensor_tensor(
                            out_tiles[db][:], po[:], bo[:, db:db + 1],
                            _f(resid[db][:]), op0=AL.add, op1=AL.add)

                with tc.tile_pool(name="x2p", bufs=1) as x2p:
                    x2 = [x2p.tile([P, TOK], F32R, tag=f"x2_{d}", name=f"x2_{d}")
                          for d in range(NB)]

                    with tc.tile_pool(name="x1p", bufs=1) as x1p:
                        x1 = [x1p.tile([P, TOK], F32R, tag=f"x1_{d}",
                                       name=f"x1_{d}") for d in range(NB)]

                        # ---------------- self-attention ----------------
                        with (
                            tc.tile_pool(name="sa", bufs=1) as sa_pool,
                            tc.tile_pool(name="sa_ps", bufs=1,
                                         space="PSUM") as sa_psum,
                        ):
                            xo = []
                            for d in range(NB):
                                t = sa_pool.tile([P, TOK], F32R, tag=f"xo{d}",
                                                 name=f"xo{d}")
                                nc.sync.dma_start(
                                    out=t[:], in_=d_xoT[d * P:(d + 1) * P, :])
                                xo.append(t)
                            xf = []
                            for d in range(NB):
                                t = sa_pool.tile([P, L], F32R, tag=f"xf{d}",
                                                 name=f"xf{d}")
                                nc.sync.dma_start(
                                    out=t[:], in_=d_xfT[d * P:(d + 1) * P, :])
                                xf.append(t)
                            attention(xo, xf, "sa", True, x1, xo,
                                      sa_pool, sa_psum)
                            layernorm_T(x1, ln_sb["g1"][:], ln_sb["b1"][:], x1,
                                        sa_pool, sa_psum)

                        # ---------------- cross-attention ----------------
                        with (
                            tc.tile_pool(name="ca", bufs=1) as ca_pool,
                            tc.tile_pool(name="ca_ps", bufs=1,
                                         space="PSUM") as ca_psum,
                        ):
                            mm = []
                            for d in range(NB):
                                t = ca_pool.tile([P, L], F32R, tag=f"mm{d}",
                                                 name=f"mm{d}")
                                nc.sync.dma_start(
                                    out=t[:], in_=d_memT[d * P:(d + 1) * P, :])
                                mm.append(t)
                            attention(x1, mm, "ca", False, x2, x1,
                                      ca_pool, ca_psum)
                            layernorm_T(x2, ln_sb["g2"][:], ln_sb["b2"][:], x2,
                                        ca_pool, ca_psum)

                    # ---------------- FFN ----------------
                    with (
                        tc.tile_pool(name="ff", bufs=1) as ff_pool,
                        tc.tile_pool(name="ff_ps", bufs=1,
                                     space="PSUM") as ff_psum,
                    ):
                        ht = []
                        for ff in range(NF):
                            w1 = ff_pool.tile([P, NB, P], F32R, tag="w1t",
                                              bufs=2)
                            nc.sync.dma_start(
                                out=w1[:],
                                in_=d_fc1[ff])
                            pf = ff_psum.tile([P, TOK], F32, tag="pf",
                                              name="pf", bufs=2)
                            for t in range(NB):
                                nc.tensor.matmul(pf[:], _r(w1[:, t, :]),
                                                 _r(x2[t][:]),
                                                 start=(t == 0),
                                                 stop=(t == NB - 1))
                            h = ff_pool.tile([P, TOK], F32R, tag=f"ht{ff}",
                                             name=f"ht{ff}")
                            nc.scalar.activation(h[:], pf[:], AF.Relu,
                                                 bias=fc1b[:, ff:ff + 1],
                                                 scale=1.0)
                            ht.append(h)
                        x3 = [ff_pool.tile([P, TOK], F32R, tag=f"x3_{d}",
                                           name=f"x3_{d}")
                              for d in range(NB)]
                        for db in range(NB):
                            w2 = ff_pool.tile([P, NF, P], F32R, tag="w2t",
                                              bufs=2)
                            nc.sync.dma_start(
                                out=w2[:],
                                in_=d_fc2[db])
                            pf2 = ff_psum.tile([P, TOK], F32, tag="pf2",
                                               name="pf2", bufs=2)
                            for t in range(NF):
                                nc.tensor.matmul(pf2[:], _r(w2[:, t, :]),
                                                 _r(ht[t][:]),
                                                 start=(t == 0),
                                                 stop=(t == NF - 1))
                            nc.vector.scalar_tensor_tensor(
                                x3[db][:], pf2[:], fc2b[:, db:db + 1],
                                _f(x2[db][:]), op0=AL.add, op1=AL.add)
                        layernorm_T(x3, ln_sb["g3"][:], ln_sb["b3"][:], x3,
                                    ff_pool, ff_psum)

                        outsb = [ff_pool.tile([P, D], F32, tag=f"os{s}",
                                              name=f"os{s}")
                                 for s in range(SLOTS)]
                        for db in range(NB):
                            for s in range(SLOTS):
                                ptr = ff_psum.tile([P, P], F32, tag="ptr",
                                                   name="ptr", bufs=2)
                                nc.tensor.matmul(
                                    _r(ptr[:]),
                                    _r(x3[db][:, s * P:(s + 1) * P]),
                                    _r(ident[:]), is_transpose=True)
                                nc.vector.tensor_copy(
                                    outsb[s][:, db * P:(db + 1) * P],
                                    ptr[:])
                        for s in range(SLOTS):
                            nc.sync.dma_start(
                                out=d_out[s * P:(s + 1) * P, :],
                                in_=outsb[s][:])

    nc.finalize()
    return nc


@functools.lru_cache(maxsize=4)
def _get_nc(reps=1):
    return _build_nc(reps)


def _rel_bucket_np(v):
    """T5 causal bucket for relative distance v = q - k (>= 0)."""
    n = np.maximum(v, 0)
    max_exact = NUM_BUCKETS // 2
    nf = np.maximum(n.astype(np.float32), 1.0)
    val_large = max_exact + (
        np.log(nf / max_exact) / math.log(MAX_DISTANCE / max_exact)
        * (NUM_BUCKETS - max_exact)
    ).astype(np.int32)
    val_large = np.minimum(val_large, NUM_BUCKETS - 1)
    return np.where(n < max_exact, n, val_large).astype(np.int32)


def _build_bc(rel_emb, g):
    """BC[h, i, w] = 8*band_h((w - 128 + 128 g) - i); -480 where q < k."""
    v = (np.arange(1024)[None, :] - 128 + 128 * g) - np.arange(P)[:, None]
    bucket = _rel_bucket_np(v)                      # [128, 1024]
    band = rel_emb[bucket]                          # [128, 1024, 16]
    band = 8.0 * np.transpose(band, (2, 0, 1))      # [16, 128, 1024]
    band[:, v < 0] = MASK8
    out = np.zeros((H, P, 1280), dtype=np.float32)
    out[:, :, :1024] = band
    return out


def _rearr_bias(b):
    return np.ascontiguousarray(b.reshape(-1, P).T, dtype=np.float32)


def _make_in_maps(inp):
    x = np.asarray(inp["x"], np.float32)
    mem = np.asarray(inp["mem"], np.float32)
    rel_emb = np.asarray(inp["rel_emb"], np.float32)

    def _tile4(w, ko, mo):
        # w [K, M] -> [M//128/mo? ...] -> [mblk, p, kblk, 128]
        kb, mb = w.shape[0] // P, w.shape[1] // P
        return np.ascontiguousarray(
            w.reshape(kb, P, mb, P).transpose(2, 1, 0, 3), np.float32)

    shared = {}
    for k in ("sa_wq", "sa_wk", "sa_wv", "sa_wo",
              "ca_wq", "ca_wk", "ca_wv", "ca_wo", "fc1_w", "fc2_w"):
        shared[k] = _tile4(np.asarray(inp[k]), None, None)
    for k in ("sa_bq", "sa_bk", "sa_bv", "sa_bo",
              "ca_bq", "ca_bk", "ca_bv", "ca_bo", "fc1_b", "fc2_b",
              "ln1_g", "ln1_b", "ln2_g", "ln2_b", "ln3_g", "ln3_b"):
        shared[k] = _rearr_bias(np.asarray(inp[k]))
    bc = [_build_bc(rel_emb, g) for g in range(2)]

    in_maps = []
    for c in range(8):
        b, g = c // 2, c % 2
        rows = np.concatenate(
            [x[b, (2 * s + g) * P:(2 * s + g + 1) * P] for s in range(SLOTS)])
        m = dict(shared)
        m["x_ownT"] = np.ascontiguousarray(rows.T, np.float32)
        m["x_fullT"] = np.ascontiguousarray(x[b].T, np.float32)
        m["memT"] = np.ascontiguousarray(mem[b].T, np.float32)
        m["bc"] = bc[g]
        in_maps.append(m)
    return in_maps


def kernel(x, mem, tgt_mask, mem_mask,
           sa_wq, sa_bq, sa_wk, sa_bk, sa_wv, sa_bv, sa_wo, sa_bo, rel_emb,
           ca_wq, ca_bq, ca_wk, ca_bk, ca_wv, ca_bv, ca_wo, ca_bo,
           fc1_w, fc1_b, fc2_w, fc2_b,
           ln1_g, ln1_b, ln2_g, ln2_b, ln3_g, ln3_b, _trace=False):
    nc = _get_nc()
    in_maps = _make_in_maps(dict(
        x=x, mem=mem, rel_emb=rel_emb,
        sa_wq=sa_wq, sa_wk=sa_wk, sa_wv=sa_wv, sa_wo=sa_wo,
        sa_bq=sa_bq, sa_bk=sa_bk, sa_bv=sa_bv, sa_bo=sa_bo,
        ca_wq=ca_wq, ca_wk=ca_wk, ca_wv=ca_wv, ca_wo=ca_wo,
        ca_bq=ca_bq, ca_bk=ca_bk, ca_bv=ca_bv, ca_bo=ca_bo,
        fc1_w=fc1_w, fc1_b=fc1_b, fc2_w=fc2_w, fc2_b=fc2_b,
        ln1_g=ln1_g, ln1_b=ln1_b, ln2_g=ln2_g, ln2_b=ln2_b,
        ln3_g=ln3_g, ln3_b=ln3_b))

    res = run_bass_kernel_spmd(nc, in_maps, list(range(8)), trace=_trace)
    out = np.empty((B, L, D), np.float32)
    for c in range(8):
        b, g = c // 2, c % 2
        oc = res.results[c]["out_own"]
        for s in range(SLOTS):
            out[b, (2 * s + g) * P:(2 * s + g + 1) * P] = \
                oc[s * P:(s + 1) * P]
    kernel.last_exec_time_ns = res.exec_time_ns
    return out



# revision 73
# speedup vs baseline: 1124.1772x; 1124.1772x over previous
"""Trainium2 Bass kernel for a T5-style decoder layer (self-attn with causal
rel-pos bias, cross-attn, FFN, 3 post-LNs).

Sharding: 8 cores = (batch b in 0..3) x (parity g in 0..1). Core (b, g) owns
query blocks {g, 2+g, 4+g, 6+g} (128 rows each) of batch b and computes the
full layer for those 512 rows. K/V work is duplicated across the pair; no
cross-core collectives are needed. Causal score work is padded to a uniform
(2,4,6,8) key-block pattern so one SPMD program serves all cores; padded
blocks are killed by the exp-band (host bakes exp(bias+mask), 0 where
masked).

v2 design notes (vs v0 baseline):
- all matmul *moving* operands are bf16 (cycles/row = 1.0 at any free size);
  residual stream x1/x2/x3 stays f32r for accuracy.
- rel-pos bias enters as EB = exp(band) bf16, applied by a DVE multiply on
  the exp'd scores (2x DVE mode) instead of an identity-matmul accumulate.
- K/V biases dropped: bk shifts every score of a query equally (softmax
  invariant); bv contributes bv@wo to the output (folded into bo on host).
- V^T is built directly (out[key, vdim] = sum_d x[d,key] * wv[d,vdim]) per
  hp-pair with free=256, killing the per-head PE transposes.
- LN: out = x*(g (x) rsd) + R with rank-2 R = g (x) nmr + b (x) 1 computed by
  one PE matmul per block; mean/var reductions interleaved into the Wo / fc2
  loops; reciprocal_approx_fast everywhere.
- weight/EB/mem prefetch via rotating pools so the PE never waits on DMA at
  phase boundaries.
"""

import functools
import math
from collections import deque

import ml_dtypes
import numpy as np

import concourse.bass as bass
import concourse.bacc as bacc
import concourse.mybir as mybir
import concourse.tile as tile
from concourse.bass_utils import run_bass_kernel_spmd
from concourse.masks import make_identity

F32 = mybir.dt.float32
F32R = mybir.dt.float32r
BF16 = mybir.dt.bfloat16
AL = mybir.AluOpType
AF = mybir.ActivationFunctionType

B, L, D, H, DK, DFF = 4, 1024, 1024, 16, 64, 4096
P = 128
NB = D // P            # 8 d_model blocks
NF = DFF // P          # 32 d_ff blocks
TOK = 512              # tokens owned per core
SLOTS = 4              # query blocks of 128 per core
NPAIR = 4              # hp pairs (each pair = 4 heads = 256 qkv dims)
NUM_BUCKETS, MAX_DISTANCE = 32, 128
EPS = 1e-5


def _r(x):
    return x.bitcast(F32R)


def _f(x):
    return x.bitcast(F32)


def _build_nc(reps=1, dbg=None):
    nc = bacc.Bacc(trn_type="TRN2")

    def inp(name, shape, dt=F32):
        return nc.declare_dram_parameter(name, list(shape), dt, isOutput=False)

    d_xo = inp("xoT", (P, NB, TOK), BF16)   # own tokens: Q moving + residual
    d_xf = inp("xfT", (P, NB, L), BF16)     # all tokens, self K/V source
    d_mem = inp("memT", (P, NB, L), BF16)   # memory, cross K/V source
    d_eb = inp("eb", (NB, P, 2, 1280), BF16)  # exp(band), paired per hp
    d_ball = inp("bias_all", (P, 96))        # packed per-partition consts
    d_gball = inp("gb_all", (1, 6, NB, P), BF16)  # LN g/b rows
    dw = {}
    for pre in ("sa", "ca"):
        for nm in ("wq", "wk", "wo"):
            dw[f"{pre}_{nm}"] = inp(f"{pre}_{nm}", (NB, P, NB, P), BF16)
        dw[f"{pre}_wv2"] = inp(f"{pre}_wv2", (NPAIR, P, NB, 256), BF16)
    d_fc1 = inp("fc1_w", (NF, P, NB, P), BF16)
    d_fc2 = inp("fc2_w", (NB, P, NF, P), BF16)
    d_out = nc.declare_dram_parameter("out_own", [TOK, D], F32, isOutput=True)

    with (
        nc.allow_low_precision(reason="bf16 matmul streams; tol 2e-2"),
        tile.TileContext(nc) as tc,
    ):
        with tc.tile_pool(name="persist", bufs=1) as pers:
            identf = pers.tile([P, P], F32, tag="identf")
            make_identity(nc, identf[:])
            ident = pers.tile([P, P], F32R, tag="ident")
            nc.scalar.copy(out=ident[:], in_=identf[:])
            ones_col = pers.tile([P, 1], F32R, tag="ones")
            nc.gpsimd.memset(_f(ones_col[:]), 1.0)
            ones_bf = pers.tile([P, 1], BF16, tag="onesb")
            nc.gpsimd.memset(ones_bf[:], 1.0)
            ones_row = pers.tile([1, P], F32R, tag="onesr")
            nc.gpsimd.memset(_f(ones_row[:]), 1.0)
            ones_row_bf = pers.tile([1, P], BF16, tag="onesrb")
            nc.gpsimd.memset(ones_row_bf[:], 1.0)
            eps_t = pers.tile([1, 1], F32, tag="epsc")
            nc.gpsimd.memset(eps_t[:], EPS)

            ball = pers.tile([P, 96], F32, tag="ball")
            nc.sync.dma_start(out=ball[:], in_=d_ball[:, :])
            gball = pers.tile([1, 6, NB, P], BF16, tag="gball")
            nc.sync.dma_start(out=gball[:], in_=d_gball[:, :, :, :])
            bias_sb = {
                "sa_bq": ball[:, 0:8], "sa_bo": ball[:, 8:16],
                "ca_bq": ball[:, 16:24], "ca_bo": ball[:, 24:32],
            }
            fc1b = ball[:, 32:64]
            fc2b = ball[:, 64:72]
            ln_sb = {}
            for ii, i in enumerate(("1", "2", "3")):
                ln_sb[f"g{i}"] = ball[:, 72 + 8 * ii:80 + 8 * ii]
                ln_sb[f"gr{i}"] = gball[:, 2 * ii, :, :]
                ln_sb[f"br{i}"] = gball[:, 2 * ii + 1, :, :]

            for _rep in range(reps):
                with tc.tile_pool(name="outer", bufs=1) as outer:
                    x2 = [outer.tile([P, TOK], BF16, tag=f"x2_{d}",
                                     name=f"x2_{d}") for d in range(NB)]
                    nmr_t = outer.tile([1, TOK], BF16, tag="nmr")
                    ones_tok = outer.tile([1, TOK], BF16, tag="onestok")
                    nc.gpsimd.memset(ones_tok[:], 1.0)
                    rsd_t = outer.tile([1, TOK], F32, tag="rsd")
                    rsd_bf = outer.tile([1, TOK], BF16, tag="rsdb")
                    rsdr = outer.tile([P, TOK], F32, tag="rsdr")

                    # ---- layernorm helpers (head interleaved with caller
                    # loop via ln_head(db); tail emits per-block outputs) ----
                    def ln_head(src, psum, acc_tag, state, db, bf=True):
                        ones = ones_bf if bf else ones_col
                        if db == 0:
                            state["pm"] = psum.tile([1, TOK], F32, tag=acc_tag,
                                                    name="pm", bufs=2)
                            state["pv2"] = psum.tile([1, TOK], F32,
                                                     tag=acc_tag, name="pv2",
                                                     bufs=2)
                        nc.tensor.matmul(state["pm"][:], ones[:],
                                         src[db][:],
                                         start=(db == 0), stop=(db == NB - 1))
                        if bf:
                            sq = outer.tile([P, TOK], BF16, tag="sqb", bufs=2)
                            nc.scalar.square(sq[:], src[db][:])
                        else:
                            sq = outer.tile([P, TOK], F32R, tag="sq", bufs=2)
                            nc.scalar.square(sq[:], _f(src[db][:]))
                        nc.tensor.matmul(state["pv2"][:], ones[:],
                                         sq[:],
                                         start=(db == 0), stop=(db == NB - 1))

                    def ln_tail(src, gi, psum, r_tag, bc_tag, state,
                                post_blk=None, r_bufs=2, bf=True):
                        g_ap = ln_sb[f"g{gi}"]
                        gr_ap = ln_sb[f"gr{gi}"]
                        br_ap = ln_sb[f"br{gi}"]
                        pm, pv2 = state["pm"], state["pv2"]
                        mu = outer.tile([1, TOK], F32, tag="mu")
                        nc.vector.tensor_scalar_mul(mu[:], pm[:], 1.0 / D)
                        musq = outer.tile([1, TOK], F32, tag="musq")
                        nc.vector.tensor_mul(musq[:], mu[:], mu[:])
                        var = outer.tile([1, TOK], F32, tag="var")
                        nc.vector.scalar_tensor_tensor(
                            var[:], pv2[:], 1.0 / D, musq[:],
                            op0=AL.mult, op1=AL.subtract)
                        std = outer.tile([1, TOK], F32, tag="std")
                        nc.scalar.activation(std[:], var[:], AF.Sqrt,
                                             bias=eps_t[:])
                        std_c = outer.tile([1, TOK], F32, tag="stdc")
                        nc.vector.tensor_copy(std_c[:], std[:])
                        nc.vector.reciprocal_approx_fast(rsd_t[:], std_c[:])
                        nc.vector.tensor_copy(rsd_bf[:], rsd_t[:])
                        nc.vector.scalar_tensor_tensor(
                            nmr_t[:], mu[:], -1.0, rsd_t[:],
                            op0=AL.mult, op1=AL.mult)
                        pbc = psum.tile([P, TOK], F32, tag=bc_tag, name="pbc",
                                        bufs=r_bufs)
                        nc.tensor.matmul(pbc[:], ones_row_bf[:],
                                         rsd_bf[:])
                        nc.scalar.copy(out=rsdr[:], in_=pbc[:])
                        for db in range(NB):
                            pr = psum.tile([P, TOK], F32, tag=r_tag, name="pr",
                                           bufs=r_bufs)
                            nc.tensor.matmul(pr[:], gr_ap[:, db, :],
                                             nmr_t[:],
                                             start=True, stop=False)
                            nc.tensor.matmul(pr[:], br_ap[:, db, :],
                                             ones_tok[:],
                                             start=False, stop=True)
                            t1 = outer.tile([P, TOK], F32, tag="t1", bufs=2)
                            nc.vector.scalar_tensor_tensor(
                                t1[:], src[db][:] if bf else _f(src[db][:]),
                                g_ap[:, db:db + 1],
                                rsdr[:], op0=AL.mult, op1=AL.mult)
                            nc.vector.tensor_add(src[db][:], t1[:], pr[:])
                            if post_blk is not None:
                                post_blk(db)

                    # ================= attention =================
                    def attention(q_src, kvT, pre, causal, out_tiles, resid,
                                  pool, psum, early_dmas=None, dbg_stash=None,
                                  q_all=False):
                        wqv, wkv = dw[f"{pre}_wq"], dw[f"{pre}_wk"]
                        wvv, wov = dw[f"{pre}_wv2"], dw[f"{pre}_wo"]
                        bq, bo = bias_sb[f"{pre}_bq"], bias_sb[f"{pre}_bo"]
                        AO = [pool.tile([P, TOK], BF16, tag=f"ao{hp}",
                                        name=f"ao{hp}", bufs=1)
                              for hp in range(NB)]

                        def fetch(p, with_q=True):
                            ws = {}
                            for j in range(2):
                                hp = 2 * p + j
                                if with_q:
                                    wq_t = wpool.tile([P, NB, P], BF16,
                                                      tag="wqt", bufs=8)
                                    nc.sync.dma_start(out=wq_t[:],
                                                      in_=wqv[hp])
                                    ws[f"wq{j}"] = wq_t
                                wk_t = wpool.tile([P, NB, P], BF16, tag="wkt",
                                                  bufs=4)
                                nc.sync.dma_start(out=wk_t[:], in_=wkv[hp])
                                ws[f"wk{j}"] = wk_t
                                if causal:
                                    eb = pool.tile([P, 2, 1280], BF16,
                                                   tag="ebt", bufs=2)
                                    nc.sync.dma_start(out=eb[:],
                                                      in_=d_eb[hp])
                                    ws[f"eb{j}"] = eb
                            wv_t = wpool.tile([P, NB, 256], BF16, tag="wvt",
                                              bufs=2)
                            nc.sync.dma_start(out=wv_t[:], in_=wvv[p])
                            ws["wv"] = wv_t
                            return ws

                        q_all_sb = []
                        if q_all:
                            wq_ts = []
                            for hp in range(NB):
                                wq_t = wpool.tile([P, NB, P], BF16,
                                                  tag="wqt", bufs=8)
                                nc.sync.dma_start(out=wq_t[:], in_=wqv[hp])
                                wq_ts.append(wq_t)
                            if early_dmas is not None:
                                early_dmas()
                            cur = fetch(0, with_q=False)
                            for hp in range(NB):
                                pq = psum.tile([P, TOK], F32, tag="ps",
                                               name="pq", bufs=3)
                                for t in range(NB):
                                    nc.tensor.matmul(
                                        pq[:], wq_ts[hp][:, t, :],
                                        q_src[t][:],
                                        start=(t == 0), stop=(t == NB - 1))
                                qs = pool.tile([P, TOK], BF16, tag="qsb",
                                               name="qsf", bufs=8)
                                nc.vector.tensor_scalar_add(
                                    qs[:], pq[:], bq[:, hp:hp + 1])
                                q_all_sb.append(qs)
                        else:
                            cur = fetch(0)
                            if early_dmas is not None:
                                early_dmas()

                        pend = []   # deferred normalize from previous pair
                        for p in range(NPAIR):
                            nxt = (fetch(p + 1, with_q=not q_all)
                                   if p + 1 < NPAIR else None)
                            # flush previous pair's normalize (recips done)
                            for fn in pend:
                                fn()
                            pend = []

                            # ---- projections ----
                            q_sb, k_sb, vh = [], [], []

                            def do_q():
                                if q_all:
                                    q_sb.extend(q_all_sb[2 * p:2 * p + 2])
                                    return
                                for j in range(2):
                                    hp = 2 * p + j
                                    pq = psum.tile([P, TOK], F32, tag="ps",
                                                   name="pq", bufs=3)
                                    for t in range(NB):
                                        nc.tensor.matmul(
                                            pq[:], cur[f"wq{j}"][:, t, :],
                                            q_src[t][:],
                                            start=(t == 0),
                                            stop=(t == NB - 1))
                                    qs = pool.tile([P, TOK], BF16, tag="qsb",
                                                   bufs=8)
                                    nc.vector.tensor_scalar_add(
                                        qs[:], pq[:], bq[:, hp:hp + 1])
                                    q_sb.append(qs)
                                    if dbg_stash is not None and \
                                            dbg_stash[0] == "q":
                                        dt_ = pool.tile([P, TOK], BF16,
                                                        tag=f"dq{hp}",
                                                        name=f"dq{hp}")
                                        nc.vector.tensor_copy(dt_[:], qs[:])
                                        dbg_stash[1].append(dt_)

                            def do_kv():
                                for j in range(2):
                                    ks = pool.tile([P, L], BF16, tag="ksb",
                                                   bufs=2)
                                    for half in range(2):
                                        sl = slice(half * 512,
                                                   (half + 1) * 512)
                                        pk = psum.tile([P, 512], F32,
                                                       tag="pkv", name="pk",
                                                       bufs=2)
                                        for t in range(NB):
                                            nc.tensor.matmul(
                                                pk[:], cur[f"wk{j}"][:, t, :],
                                                kvT[t][:, sl],
                                                start=(t == 0),
                                                stop=(t == NB - 1))
                                        nc.vector.tensor_copy(ks[:, sl],
                                                              pk[:])
                                    k_sb.append(ks)
                                    if dbg_stash is not None and \
                                            dbg_stash[0] == "k":
                                        hp = 2 * p + j
                                        dt_ = pool.tile([P, TOK], BF16,
                                                        tag=f"dk{hp}",
                                                        name=f"dk{hp}")
                                        nc.vector.tensor_copy(
                                            dt_[:], ks[:, 0:TOK])
                                        dbg_stash[1].append(dt_)
                                # V^T direct: vh[kb][key, head_in_pair, 0:64]
                                for kb in range(8):
                                    vt = pool.tile([P, 4, 65], BF16,
                                                   tag=f"vh{kb}",
                                                   name=f"vh{kb}", bufs=1)
                                    nc.gpsimd.memset(vt[:], 1.0)
                                    pvt = psum.tile([P, 256], F32, tag="pva",
                                                    name="pvt", bufs=3)
                                    for t in range(NB):
                                        nc.tensor.matmul(
                                            pvt[:],
                                            kvT[t][:, kb * P:(kb + 1) * P],
                                            cur["wv"][:, t, :],
                                            start=(t == 0),
                                            stop=(t == NB - 1))
                                    nc.vector.tensor_copy(
                                        vt[:, :, 0:64],
                                        pvt[:].rearrange("p (h c) -> p h c",
                                                         c=64))
                                    vh.append(vt)

                            if causal:
                                do_q()
                                do_kv()
                            else:
                                do_kv()
                                do_q()
                            if dbg_stash is not None and \
                                    dbg_stash[0] in ("q", "k"):
                                cur = nxt
                                continue

                            # ---- scores / softmax / AV ----
                            pts_hh = {}
                            pav_hh = {}
                            rec_hh = {}

                            def scores(hh):
                                j, lo = hh // 2, (hh % 2) * 64
                                hsl = slice(lo, lo + 64)
                                pts = []
                                for kb in range(8):
                                    smin = kb // 2 if causal else 0
                                    n = TOK - smin * P
                                    ns = SLOTS - smin
                                    ps = psum.tile([P, TOK], F32, tag="ps",
                                                   name="ps", bufs=3)
                                    nc.tensor.matmul(
                                        ps[:, 0:n],
                                        k_sb[j][hsl, kb * P:(kb + 1) * P],
                                        q_sb[j][hsl, smin * P:TOK])
                                    if causal:
                                        pe = pool.tile([P, TOK], BF16,
                                                       tag="pe", bufs=2)
                                        nc.scalar.activation(
                                            pe[:, 0:n], ps[:, 0:n], AF.Exp,
                                            scale=0.125)
                                        w0 = 256 * smin - 128 * kb + 128
                                        ebv = cur[f"eb{j}"][
                                            :, hh % 2,
                                            w0:w0 + ns * 256].rearrange(
                                            "p (s c) -> p s c",
                                            c=256)[:, :, 0:P]
                                        pt = pool.tile([P, TOK], BF16,
                                                       tag="pt", bufs=16)
                                        nc.vector.tensor_mul(
                                            pt[:, 0:n].rearrange(
                                                "p (s c) -> p s c", c=P),
                                            pe[:, 0:n].rearrange(
                                                "p (s c) -> p s c", c=P),
                                            ebv)
                                    else:
                                        pt = pool.tile([P, TOK], BF16,
                                                       tag="pe", bufs=16)
                                        nc.scalar.activation(
                                            pt[:, 0:n], ps[:, 0:n], AF.Exp,
                                            scale=0.125)
                                    pts.append(pt)
                                pts_hh[hh] = pts

                            def pav_f(hh):
                                pav = psum.tile([65, TOK], F32, tag="pva",
                                                name="pav", bufs=3)
                                for kb in range(8):
                                    smin = kb // 2 if causal else 0
                                    n = TOK - smin * P
                                    nc.tensor.matmul(
                                        pav[:, smin * P:TOK],
                                        vh[kb][:, hh, :],
                                        pts_hh[hh][kb][:, 0:n],
                                        start=(kb == 0), stop=(kb == 7))
                                # copy den to SBUF on DVE first: the custom
                                # DVE recip lacks cross-engine dep tracking,
                                # in-order DVE queue makes this safe
                                dsb = pool.tile([1, TOK], F32, tag="dens",
                                                bufs=2)
                                nc.vector.tensor_copy(dsb[:], pav[64:65, :])
                                rec = pool.tile([1, TOK], F32, tag="rec",
                                                bufs=2)
                                nc.vector.reciprocal_approx_fast(
                                    rec[:], dsb[:])
                                rcb = pool.tile([1, TOK], BF16, tag="recb",
                                                bufs=2)
                                nc.vector.tensor_copy(rcb[:], rec[:])
                                pav_hh[hh] = pav
                                rec_hh[hh] = rcb
                                if dbg_stash is not None and \
                                        dbg_stash[0] == "pav" and \
                                        p == 0 and hh == 0:
                                    dnum = pool.tile([64, TOK], F32,
                                                     tag="dnum")
                                    nc.vector.tensor_copy(dnum[:],
                                                          pav[0:64, :])
                                    dden = pool.tile([1, TOK], F32,
                                                     tag="dden")
                                    nc.vector.tensor_copy(dden[:],
                                                          pav[64:65, :])
                                    nc.sync.dma_start(
                                        out=d_out[0:64, 0:TOK],
                                        in_=dnum[:])
                                    nc.sync.dma_start(
                                        out=d_out[64:65, 0:TOK],
                                        in_=dden[:])
                                    nc.sync.dma_start(
                                        out=d_out[65:66, 0:TOK],
                                        in_=rec[:])

                            def norm_f(hh, hp, hsl):
                                def run():
                                    prr = psum.tile([64, TOK], F32, tag="ps",
                                                    name="prr", bufs=3)
                                    nc.tensor.matmul(
                                        prr[:], ones_row_bf[0:1, 0:64],
                                        rec_hh[hh][:])
                                    rrep = pool.tile([64, TOK], F32,
                                                     tag="rrep", bufs=2)
                                    nc.scalar.copy(out=rrep[:], in_=prr[:])
                                    nc.vector.tensor_mul(
                                        AO[hp][hsl, :], pav_hh[hh][0:64, :],
                                        rrep[:])
                                return run

                            scores(0)
                            scores(1)
                            pav_f(0)
                            scores(2)
                            pav_f(1)
                            norm_f(0, 2 * p, slice(0, 64))()
                            scores(3)
                            pav_f(2)
                            norm_f(1, 2 * p, slice(64, 128))()
                            pav_f(3)
                            pend = [norm_f(2, 2 * p + 1, slice(0, 64)),
                                    norm_f(3, 2 * p + 1, slice(64, 128))]
                            cur = nxt
                        for fn in pend:
                            fn()
                        if dbg_stash is not None:
                            attention.last_ao = AO
                            return {}

                        # ---- Wo + residual (+ LN head interleaved) ----
                        st = {}
                        wot = deque()

                        def fetch_wo(db):
                            t = wpool.tile([P, NB, P], BF16, tag="wot",
                                           bufs=2)
                            nc.sync.dma_start(out=t[:], in_=wov[db])
                            wot.append(t)

                        fetch_wo(0)
                        fetch_wo(1)
                        for db in range(NB):
                            if db + 2 < NB:
                                fetch_wo(db + 2)
                            w = wot.popleft()
                            po = psum.tile([P, TOK], F32, tag="ps", name="po",
                                           bufs=3)
                            for hp in range(NB):
                                nc.tensor.matmul(po[:], w[:, hp, :],
                                                 AO[hp][:],
                                                 start=(hp == 0),
                                                 stop=(hp == NB - 1))
                            rin = resid[db][:]
                            if rin.dtype == F32R:
                                rin = _f(rin)
                            nc.vector.scalar_tensor_tensor(
                                out_tiles[db][:], po[:], bo[:, db:db + 1],
                                rin, op0=AL.add, op1=AL.add)
                            ln_head(out_tiles, psum, "pkv", st, db)
                        attention.last_ao = AO
                        return st

                    with (
                        tc.tile_pool(name="x1mm", bufs=1) as x1mm,
                        tc.tile_pool(name="wp", bufs=1) as wpool,
                    ):
                        x1 = [x1mm.tile([P, TOK], BF16, tag=f"x1_{d}",
                                        name=f"x1_{d}") for d in range(NB)]
                        mm_t = x1mm.tile([P, NB, L], BF16, tag="mmall")
                        mm = [mm_t[:, t, :] for t in range(NB)]

                        # ------------- self-attention -------------
                        with (
                            tc.tile_pool(name="sa", bufs=1) as sa_pool,
                            tc.tile_pool(name="sa_ps", bufs=1,
                                         space="PSUM") as sa_psum,
                        ):
                            xo_t = sa_pool.tile([P, NB, TOK], BF16,
                                                tag="xoall")
                            nc.sync.dma_start(out=xo_t[:], in_=d_xo[:, :, :])
                            xo = [xo_t[:, t, :] for t in range(NB)]
                            xf_t = sa_pool.tile([P, NB, L], BF16,
                                                tag="xfall")
                            xf = [xf_t[:, t, :] for t in range(NB)]

                            def early():
                                nc.sync.dma_start(out=xf_t[:],
                                                  in_=d_xf[:, :, :])
                                nc.sync.dma_start(out=mm_t[:],
                                                  in_=d_mem[:, :, :])

                            stash = ([dbg, []]
                                     if dbg in ("q", "k", "pav") else None)
                            st = attention(xo, xf, "sa", True, x1, xo,
                                           sa_pool, sa_psum, early_dmas=early,
                                           dbg_stash=stash, q_all=True)
                            if dbg not in ("x1pre", "q", "k", "pav"):
                                ln_tail(x1, "1", sa_psum, "ps", "pva", st,
                                        r_bufs=3)
                            if dbg in ("x1pre", "x1", "ao", "q", "k"):
                                dsrc = (attention.last_ao if dbg == "ao"
                                        else stash[1] if stash else x1)
                                identb = sa_pool.tile([P, P], BF16,
                                                      tag="identb")
                                nc.scalar.copy(out=identb[:], in_=identf[:])
                                osb = [sa_pool.tile([P, D], F32,
                                                    tag=f"dos{s}",
                                                    name=f"dos{s}")
                                       for s in range(SLOTS)]
                                for db in range(NB):
                                    for s in range(SLOTS):
                                        pd = sa_psum.tile([P, P], BF16,
                                                          tag="ps", bufs=3)
                                        nc.tensor.matmul(
                                            pd[:],
                                            dsrc[db][:, s * P:(s + 1) * P],
                                            identb[:], is_transpose=True)
                                        nc.vector.tensor_copy(
                                            osb[s][:, db * P:(db + 1) * P],
                                            pd[:])
                                for s in range(SLOTS):
                                    nc.sync.dma_start(
                                        out=d_out[s * P:(s + 1) * P, :],
                                        in_=osb[s][:])

                        # ------------- cross-attention -------------
                        if dbg is None:
                            with (
                                tc.tile_pool(name="ca", bufs=1) as ca_pool,
                                tc.tile_pool(name="ca_ps", bufs=1,
                                             space="PSUM") as ca_psum,
                            ):
                                st = attention(x1, mm, "ca", False, x2, x1,
                                               ca_pool, ca_psum)
                                ln_tail(x2, "2", ca_psum, "ps", "pva", st,
                                        r_bufs=3)

                    # ---------------- FFN ----------------
                    if dbg is not None:
                        continue
                    with (
                        tc.tile_pool(name="ff", bufs=1) as ff_pool,
                        tc.tile_pool(name="ff_ps", bufs=1,
                                     space="PSUM") as ff_psum,
                    ):
                        w1q = deque()

                        def fetch_w1(ff):
                            t = ff_pool.tile([P, NB, P], BF16, tag="w1t",
                                             bufs=3)
                            nc.sync.dma_start(out=t[:], in_=d_fc1[ff])
                            w1q.append(t)

                        fetch_w1(0)
                        fetch_w1(1)
                        ht = []
                        for ff in range(NF):
                            if ff + 2 < NF:
                                fetch_w1(ff + 2)
                            w1 = w1q.popleft()
                            pf = ff_psum.tile([P, TOK], F32, tag="pf",
                                              name="pf", bufs=2)
                            for t in range(NB):
                                nc.tensor.matmul(pf[:], w1[:, t, :],
                                                 x2[t][:],
                                                 start=(t == 0),
                                                 stop=(t == NB - 1))
                            h = ff_pool.tile([P, TOK], BF16, tag=f"ht{ff}",
                                             name=f"ht{ff}")
                            nc.scalar.activation(h[:], pf[:], AF.Relu,
                                                 bias=fc1b[:, ff:ff + 1],
                                                 scale=1.0)
                            ht.append(h)
                        x3 = [ff_pool.tile([P, TOK], F32R, tag=f"x3_{d}",
                                           name=f"x3_{d}")
                              for d in range(NB)]
                        w2q = deque()

                        def fetch_w2(db):
                            t = ff_pool.tile([P, NF, P], BF16, tag="w2t",
                                             bufs=2)
                            nc.sync.dma_start(out=t[:], in_=d_fc2[db])
                            w2q.append(t)

                        fetch_w2(0)
                        fetch_w2(1)
                        st = {}
                        for db in range(NB):
                            if db + 2 < NB:
                                fetch_w2(db + 2)
                            w2 = w2q.popleft()
                            pf2 = ff_psum.tile([P, TOK], F32, tag="pf2",
                                               name="pf2", bufs=2)
                            for t in range(NF):
                                nc.tensor.matmul(pf2[:], w2[:, t, :],
                                                 ht[t][:],
                                                 start=(t == 0),
                                                 stop=(t == NF - 1))
                            nc.vector.scalar_tensor_tensor(
                                x3[db][:], pf2[:], fc2b[:, db:db + 1],
                                x2[db][:], op0=AL.add, op1=AL.add)
                            ln_head(x3, ff_psum, "pf", st, db, bf=False)

                        outsb = [ff_pool.tile([P, D], F32, tag=f"os{s}",
                                              name=f"os{s}")
                                 for s in range(SLOTS)]

                        def post_blk(db):
                            for s in range(SLOTS):
                                ptr = ff_psum.tile([P, P], F32, tag="ptr",
                                                   name="ptr", bufs=2)
                                nc.tensor.matmul(
                                    _r(ptr[:]),
                                    _r(x3[db][:, s * P:(s + 1) * P]),
                                    _r(ident[:]), is_transpose=True)
                                nc.vector.tensor_copy(
                                    outsb[s][:, db * P:(db + 1) * P],
                                    ptr[:])

                        ln_tail(x3, "3", ff_psum, "pf2", "ptr", st,
                                post_blk=post_blk, bf=False)
                        for s in range(SLOTS):
                            nc.sync.dma_start(
                                out=d_out[s * P:(s + 1) * P, :],
                                in_=outsb[s][:])

    nc.finalize()
    return nc


@functools.lru_cache(maxsize=4)
def _get_nc(reps=1, dbg=None):
    return _build_nc(reps, dbg)


def _rel_bucket_np(v):
    """T5 causal bucket for relative distance v = q - k (>= 0)."""
    n = np.maximum(v, 0)
    max_exact = NUM_BUCKETS // 2
    nf = np.maximum(n.astype(np.float32), 1.0)
    val_large = max_exact + (
        np.log(nf / max_exact) / math.log(MAX_DISTANCE / max_exact)
        * (NUM_BUCKETS - max_exact)
    ).astype(np.int32)
    val_large = np.minimum(val_large, NUM_BUCKETS - 1)
    return np.where(n < max_exact, n, val_large).astype(np.int32)


def _build_eb(rel_emb, g):
    """EB[h, i, w] = exp(band_h((w - 128 + 128 g) - i)); 0 where q < k."""
    v = (np.arange(1024)[None, :] - 128 + 128 * g) - np.arange(P)[:, None]
    bucket = _rel_bucket_np(v)                      # [128, 1024]
    band = rel_emb[bucket]                          # [128, 1024, 16]
    band = np.transpose(band, (2, 0, 1)).astype(np.float64)  # [16, 128, 1024]
    eb = np.exp(band)
    eb[:, v < 0] = 0.0
    out = np.zeros((H, P, 1280), dtype=np.float32)
    out[:, :, :1024] = eb
    return out.astype(ml_dtypes.bfloat16)


def _rearr_bias(b):
    return np.ascontiguousarray(b.reshape(-1, P).T, dtype=np.float32)


def _tile4(w, dt=ml_dtypes.bfloat16):
    kb, mb = w.shape[0] // P, w.shape[1] // P
    return np.ascontiguousarray(
        w.reshape(kb, P, mb, P).transpose(2, 1, 0, 3)).astype(dt)


def _tile_v2(w):
    """[K, M] -> [M//256, K//128, 128(k), 256(m)] for the V^T moving op."""
    kb, mb2 = w.shape[0] // P, w.shape[1] // 256
    r = w.reshape(kb, P, mb2, 256).transpose(2, 1, 0, 3)
    # want [pair, P(k_in), kb, 256] with partition dim = k_in
    return np.ascontiguousarray(r).astype(ml_dtypes.bfloat16)


def _gb_stack(g, b):
    return np.ascontiguousarray(
        np.stack([g.reshape(NB, P), b.reshape(NB, P)], axis=0)
    ).astype(ml_dtypes.bfloat16)


def _make_in_maps(inp):
    x = np.asarray(inp["x"], np.float32)
    mem = np.asarray(inp["mem"], np.float32)
    rel_emb = np.asarray(inp["rel_emb"], np.float32)

    shared = {}
    for k in ("sa_wq", "sa_wk", "sa_wo", "ca_wq", "ca_wk", "ca_wo",
              "fc1_w", "fc2_w"):
        shared[k] = _tile4(np.asarray(inp[k]))
    shared["sa_wv2"] = _tile_v2(np.asarray(inp["sa_wv"]))
    shared["ca_wv2"] = _tile_v2(np.asarray(inp["ca_wv"]))
    cols = []
    for pre in ("sa", "ca"):
        bo = np.asarray(inp[f"{pre}_bo"]) + \
            np.asarray(inp[f"{pre}_bv"]) @ np.asarray(inp[f"{pre}_wo"])
        cols.append(_rearr_bias(np.asarray(inp[f"{pre}_bq"])))
        cols.append(_rearr_bias(bo))
    # reorder: sa_bq, sa_bo, ca_bq, ca_bo
    cols = [cols[0], cols[1], cols[2], cols[3],
            _rearr_bias(np.asarray(inp["fc1_b"])),
            _rearr_bias(np.asarray(inp["fc2_b"])),
            _rearr_bias(np.asarray(inp["ln1_g"])),
            _rearr_bias(np.asarray(inp["ln2_g"])),
            _rearr_bias(np.asarray(inp["ln3_g"]))]
    shared["bias_all"] = np.ascontiguousarray(
        np.concatenate(cols, axis=1), np.float32)
    gbs = []
    for i in ("1", "2", "3"):
        gbs.append(np.asarray(inp[f"ln{i}_g"]).reshape(NB, P))
        gbs.append(np.asarray(inp[f"ln{i}_b"]).reshape(NB, P))
    shared["gb_all"] = np.ascontiguousarray(
        np.stack(gbs, axis=0)[None]).astype(ml_dtypes.bfloat16)
    eb = [_build_eb(rel_emb, g) for g in range(2)]
    # [H, P, 1280] -> [NB hp, P, 2, 1280]
    eb = [np.ascontiguousarray(
        e.reshape(NB, 2, P, 1280).transpose(0, 2, 1, 3)) for e in eb]

    def _blk(a):
        # [D, N] -> [P, NB, N]
        return np.ascontiguousarray(
            a.reshape(NB, P, a.shape[1]).transpose(1, 0, 2))

    in_maps = []
    for c in range(8):
        b, g = c // 2, c % 2
        rows = np.concatenate(
            [x[b, (2 * s + g) * P:(2 * s + g + 1) * P] for s in range(SLOTS)])
        m = dict(shared)
        m["xoT"] = _blk(np.ascontiguousarray(rows.T)).astype(
            ml_dtypes.bfloat16)
        m["xfT"] = _blk(np.ascontiguousarray(x[b].T)).astype(
            ml_dtypes.bfloat16)
        m["memT"] = _blk(np.ascontiguousarray(mem[b].T)).astype(
            ml_dtypes.bfloat16)
        m["eb"] = eb[g]
        in_maps.append(m)
    return in_maps


def kernel(x, mem, tgt_mask, mem_mask,
           sa_wq, sa_bq, sa_wk, sa_bk, sa_wv, sa_bv, sa_wo, sa_bo, rel_emb,
           ca_wq, ca_bq, ca_wk, ca_bk, ca_wv, ca_bv, ca_wo, ca_bo,
           fc1_w, fc1_b, fc2_w, fc2_b,
           ln1_g, ln1_b, ln2_g, ln2_b, ln3_g, ln3_b, _trace=False):
    nc = _get_nc()
    in_maps = _make_in_maps(dict(
        x=x, mem=mem, rel_emb=rel_emb,
        sa_wq=sa_wq, sa_wk=sa_wk, sa_wv=sa_wv, sa_wo=sa_wo,
        sa_bq=sa_bq, sa_bk=sa_bk, sa_bv=sa_bv, sa_bo=sa_bo,
        ca_wq=ca_wq, ca_wk=ca_wk, ca_wv=ca_wv, ca_wo=ca_wo,
        ca_bq=ca_bq, ca_bk=ca_bk, ca_bv=ca_bv, ca_bo=ca_bo,
        fc1_w=fc1_w, fc1_b=fc1_b, fc2_w=fc2_w, fc2_b=fc2_b,
        ln1_g=ln1_g, ln1_b=ln1_b, ln2_g=ln2_g, ln2_b=ln2_b,
        ln3_g=ln3_g, ln3_b=ln3_b))

    res = run_bass_kernel_spmd(nc, in_maps, list(range(8)), trace=_trace)
    out = np.empty((B, L, D), np.float32)
    for c in range(8):
        b, g = c // 2, c % 2
        oc = res.results[c]["out_own"]
        for s in range(SLOTS):
            out[b, (2 * s + g) * P:(2 * s + g + 1) * P] = \
                oc[s * P:(s + 1) * P]
    kernel.last_exec_time_ns = res.exec_time_ns
    return out


# revision 77
# speedup vs baseline: 1152.0173x; 1.0248x over previous
"""Trainium2 Bass kernel for a T5-style decoder layer (self-attn with causal
rel-pos bias, cross-attn, FFN, 3 post-LNs).

Sharding: 8 cores = (batch b in 0..3) x (parity g in 0..1). Core (b, g) owns
query blocks {g, 2+g, 4+g, 6+g} (128 rows each) of batch b and computes the
full layer for those 512 rows. K/V work is duplicated across the pair; no
cross-core collectives are needed. Causal score work is padded to a uniform
(2,4,6,8) key-block pattern so one SPMD program serves all cores; padded
blocks are killed by the exp-band (host bakes exp(bias+mask), 0 where
masked).

v2 design notes (vs v0 baseline):
- all matmul *moving* operands are bf16 (cycles/row = 1.0 at any free size);
  residual stream x1/x2/x3 stays f32r for accuracy.
- rel-pos bias enters as EB = exp(band) bf16, applied by a DVE multiply on
  the exp'd scores (2x DVE mode) instead of an identity-matmul accumulate.
- K/V biases dropped: bk shifts every score of a query equally (softmax
  invariant); bv contributes bv@wo to the output (folded into bo on host).
- V^T is built directly (out[key, vdim] = sum_d x[d,key] * wv[d,vdim]) per
  hp-pair with free=256, killing the per-head PE transposes.
- LN: out = x*(g (x) rsd) + R with rank-2 R = g (x) nmr + b (x) 1 computed by
  one PE matmul per block; mean/var reductions interleaved into the Wo / fc2
  loops; reciprocal_approx_fast everywhere.
- weight/EB/mem prefetch via rotating pools so the PE never waits on DMA at
  phase boundaries.
"""

import functools
import math
from collections import deque

import ml_dtypes
import numpy as np

import concourse.bass as bass
import concourse.bacc as bacc
import concourse.mybir as mybir
import concourse.tile as tile
from concourse.bass_utils import run_bass_kernel_spmd
from concourse.masks import make_identity

F32 = mybir.dt.float32
F32R = mybir.dt.float32r
BF16 = mybir.dt.bfloat16
AL = mybir.AluOpType
AF = mybir.ActivationFunctionType

B, L, D, H, DK, DFF = 4, 1024, 1024, 16, 64, 4096
P = 128
NB = D // P            # 8 d_model blocks
NF = DFF // P          # 32 d_ff blocks
TOK = 512              # tokens owned per core
SLOTS = 4              # query blocks of 128 per core
NPAIR = 4              # hp pairs (each pair = 4 heads = 256 qkv dims)
NUM_BUCKETS, MAX_DISTANCE = 32, 128
EPS = 1e-5


def _r(x):
    return x.bitcast(F32R)


def _f(x):
    return x.bitcast(F32)


def _build_nc(reps=1, dbg=None):
    nc = bacc.Bacc(trn_type="TRN2")

    def inp(name, shape, dt=F32):
        return nc.declare_dram_parameter(name, list(shape), dt, isOutput=False)

    d_xo = inp("xoT", (P, NB, TOK), BF16)   # own tokens: Q moving + residual
    d_xf = inp("xfT", (P, NB, L), BF16)     # all tokens, self K/V source
    d_mem = inp("memT", (P, NB, L), BF16)   # memory, cross K/V source
    d_eb = inp("eb", (NB, P, 2, 1280), BF16)  # exp(band), paired per hp
    d_ball = inp("bias_all", (P, 96))        # packed per-partition consts
    d_gball = inp("gb_all", (1, 6, NB, P), BF16)  # LN g/b rows
    dw = {}
    for pre in ("sa", "ca"):
        for nm in ("wq", "wk", "wo"):
            dw[f"{pre}_{nm}"] = inp(f"{pre}_{nm}", (NB, P, NB, P), BF16)
        dw[f"{pre}_wv2"] = inp(f"{pre}_wv2", (NPAIR, P, NB, 256), BF16)
    d_fc1 = inp("fc1_w", (NF, P, NB, P), BF16)
    d_fc2 = inp("fc2_w", (NB, P, NF, P), BF16)
    d_out = nc.declare_dram_parameter("out_own", [TOK, D], F32, isOutput=True)

    with (
        nc.allow_low_precision(reason="bf16 matmul streams; tol 2e-2"),
        tile.TileContext(nc) as tc,
    ):
        with tc.tile_pool(name="persist", bufs=1) as pers:
            identf = pers.tile([P, P], F32, tag="identf")
            make_identity(nc, identf[:])
            ident = pers.tile([P, P], F32R, tag="ident")
            nc.scalar.copy(out=ident[:], in_=identf[:])
            ones_col = pers.tile([P, 1], F32R, tag="ones")
            nc.gpsimd.memset(_f(ones_col[:]), 1.0)
            ones_bf = pers.tile([P, 1], BF16, tag="onesb")
            nc.gpsimd.memset(ones_bf[:], 1.0)
            ones_row = pers.tile([1, P], F32R, tag="onesr")
            nc.gpsimd.memset(_f(ones_row[:]), 1.0)
            ones_row_bf = pers.tile([1, P], BF16, tag="onesrb")
            nc.gpsimd.memset(ones_row_bf[:], 1.0)
            eps_t = pers.tile([1, 1], F32, tag="epsc")
            nc.gpsimd.memset(eps_t[:], EPS)

            ball = pers.tile([P, 96], F32, tag="ball")
            nc.sync.dma_start(out=ball[:], in_=d_ball[:, :])
            gball = pers.tile([1, 6, NB, P], BF16, tag="gball")
            nc.sync.dma_start(out=gball[:], in_=d_gball[:, :, :, :])
            bias_sb = {
                "sa_bq": ball[:, 0:8], "sa_bo": ball[:, 8:16],
                "ca_bq": ball[:, 16:24], "ca_bo": ball[:, 24:32],
            }
            fc1b = ball[:, 32:64]
            fc2b = ball[:, 64:72]
            ln_sb = {}
            for ii, i in enumerate(("1", "2", "3")):
                ln_sb[f"g{i}"] = ball[:, 72 + 8 * ii:80 + 8 * ii]
                ln_sb[f"gr{i}"] = gball[:, 2 * ii, :, :]
                ln_sb[f"br{i}"] = gball[:, 2 * ii + 1, :, :]

            for _rep in range(reps):
                with tc.tile_pool(name="outer", bufs=1) as outer:
                    x2 = [outer.tile([P, TOK], BF16, tag=f"x2_{d}",
                                     name=f"x2_{d}") for d in range(NB)]
                    nmr_t = outer.tile([1, TOK], BF16, tag="nmr")
                    ones_tok = outer.tile([1, TOK], BF16, tag="onestok")
                    nc.gpsimd.memset(ones_tok[:], 1.0)
                    rsd_t = outer.tile([1, TOK], F32, tag="rsd")
                    rsd_bf = outer.tile([1, TOK], BF16, tag="rsdb")
                    rsdr = outer.tile([P, TOK], BF16, tag="rsdr")

                    # ---- layernorm helpers (head interleaved with caller
                    # loop via ln_head(db); tail emits per-block outputs) ----
                    def ln_head(src, psum, acc_tag, state, db, bf=True):
                        ones = ones_bf if bf else ones_col
                        if db == 0:
                            state["pm"] = psum.tile([1, TOK], F32, tag=acc_tag,
                                                    name="pm", bufs=2)
                            state["pv2"] = psum.tile([1, TOK], F32,
                                                     tag=acc_tag, name="pv2",
                                                     bufs=2)
                        nc.tensor.matmul(state["pm"][:], ones[:],
                                         src[db][:],
                                         start=(db == 0), stop=(db == NB - 1))
                        if bf:
                            sq = outer.tile([P, TOK], BF16, tag="sqb", bufs=2)
                            nc.scalar.square(sq[:], src[db][:])
                        else:
                            sq = outer.tile([P, TOK], F32R, tag="sq", bufs=2)
                            nc.scalar.square(sq[:], _f(src[db][:]))
                        nc.tensor.matmul(state["pv2"][:], ones[:],
                                         sq[:],
                                         start=(db == 0), stop=(db == NB - 1))

                    def ln_tail(src, gi, psum, r_tag, bc_tag, state,
                                post_blk=None, r_bufs=2, bf=True):
                        g_ap = ln_sb[f"g{gi}"]
                        gr_ap = ln_sb[f"gr{gi}"]
                        br_ap = ln_sb[f"br{gi}"]
                        pm, pv2 = state["pm"], state["pv2"]
                        mu = outer.tile([1, TOK], F32, tag="mu")
                        nc.vector.tensor_scalar_mul(mu[:], pm[:], 1.0 / D)
                        musq = outer.tile([1, TOK], F32, tag="musq")
                        nc.vector.tensor_mul(musq[:], mu[:], mu[:])
                        var = outer.tile([1, TOK], F32, tag="var")
                        nc.vector.scalar_tensor_tensor(
                            var[:], pv2[:], 1.0 / D, musq[:],
                            op0=AL.mult, op1=AL.subtract)
                        std = outer.tile([1, TOK], F32, tag="std")
                        nc.scalar.activation(std[:], var[:], AF.Sqrt,
                                             bias=eps_t[:])
                        std_c = outer.tile([1, TOK], F32, tag="stdc")
                        nc.vector.tensor_copy(std_c[:], std[:])
                        nc.vector.reciprocal_approx_fast(rsd_t[:], std_c[:])
                        nc.vector.tensor_copy(rsd_bf[:], rsd_t[:])
                        nc.vector.scalar_tensor_tensor(
                            nmr_t[:], mu[:], -1.0, rsd_t[:],
                            op0=AL.mult, op1=AL.mult)
                        pbc = psum.tile([P, TOK], F32, tag=bc_tag, name="pbc",
                                        bufs=r_bufs)
                        nc.tensor.matmul(pbc[:], ones_row_bf[:],
                                         rsd_bf[:])
                        nc.scalar.copy(out=rsdr[:], in_=pbc[:])
                        for db in range(NB):
                            pr = psum.tile([P, TOK], F32, tag=r_tag, name="pr",
                                           bufs=r_bufs)
                            nc.tensor.matmul(pr[:], gr_ap[:, db, :],
                                             nmr_t[:],
                                             start=True, stop=False)
                            nc.tensor.matmul(pr[:], br_ap[:, db, :],
                                             ones_tok[:],
                                             start=False, stop=True)
                            if bf:
                                t1 = outer.tile([P, TOK], BF16, tag="t1b",
                                                bufs=2)
                                nc.vector.scalar_tensor_tensor(
                                    t1[:], src[db][:], g_ap[:, db:db + 1],
                                    rsdr[:], op0=AL.mult, op1=AL.mult)
                            else:
                                t1 = outer.tile([P, TOK], F32, tag="t1",
                                                bufs=2)
                                nc.vector.scalar_tensor_tensor(
                                    t1[:], _f(src[db][:]),
                                    g_ap[:, db:db + 1],
                                    rsdr[:], op0=AL.mult, op1=AL.mult)
                            nc.vector.tensor_add(src[db][:], t1[:], pr[:])
                            if post_blk is not None:
                                post_blk(db)

                    # ================= attention =================
                    def attention(q_src, kvT, pre, causal, out_tiles, resid,
                                  pool, psum, early_dmas=None, dbg_stash=None,
                                  q_all=False):
                        wqv, wkv = dw[f"{pre}_wq"], dw[f"{pre}_wk"]
                        wvv, wov = dw[f"{pre}_wv2"], dw[f"{pre}_wo"]
                        bq, bo = bias_sb[f"{pre}_bq"], bias_sb[f"{pre}_bo"]
                        AO = [pool.tile([P, TOK], BF16, tag=f"ao{hp}",
                                        name=f"ao{hp}", bufs=1)
                              for hp in range(NB)]

                        def fetch(p, with_q=True):
                            ws = {}
                            for j in range(2):
                                hp = 2 * p + j
                                if with_q:
                                    wq_t = wpool.tile([P, NB, P], BF16,
                                                      tag="wqt", bufs=8)
                                    nc.sync.dma_start(out=wq_t[:],
                                                      in_=wqv[hp])
                                    ws[f"wq{j}"] = wq_t
                                wk_t = wpool.tile([P, NB, P], BF16, tag="wkt",
                                                  bufs=4)
                                nc.sync.dma_start(out=wk_t[:], in_=wkv[hp])
                                ws[f"wk{j}"] = wk_t
                                if causal:
                                    eb = pool.tile([P, 2, 1280], BF16,
                                                   tag="ebt", bufs=2)
                                    nc.sync.dma_start(out=eb[:],
                                                      in_=d_eb[hp])
                                    ws[f"eb{j}"] = eb
                            wv_t = wpool.tile([P, NB, 256], BF16, tag="wvt",
                                              bufs=2)
                            nc.sync.dma_start(out=wv_t[:], in_=wvv[p])
                            ws["wv"] = wv_t
                            return ws

                        q_all_sb = []
                        if q_all:
                            wq_ts = []
                            for hp in range(NB):
                                wq_t = wpool.tile([P, NB, P], BF16,
                                                  tag="wqt", bufs=8)
                                nc.sync.dma_start(out=wq_t[:], in_=wqv[hp])
                                wq_ts.append(wq_t)
                            if early_dmas is not None:
                                early_dmas()
                            cur = fetch(0, with_q=False)
                            for hp in range(NB):
                                pq = psum.tile([P, TOK], F32, tag="ps",
                                               name="pq", bufs=3)
                                for t in range(NB):
                                    nc.tensor.matmul(
                                        pq[:], wq_ts[hp][:, t, :],
                                        q_src[t][:],
                                        start=(t == 0), stop=(t == NB - 1))
                                qs = pool.tile([P, TOK], BF16, tag="qsb",
                                               name="qsf", bufs=8)
                                nc.vector.tensor_scalar_add(
                                    qs[:], pq[:], bq[:, hp:hp + 1])
                                q_all_sb.append(qs)
                        else:
                            cur = fetch(0)
                            if early_dmas is not None:
                                early_dmas()

                        pend = []   # deferred normalize from previous pair
                        for p in range(NPAIR):
                            nxt = (fetch(p + 1, with_q=not q_all)
                                   if p + 1 < NPAIR else None)
                            # flush previous pair's normalize (recips done)
                            for fn in pend:
                                fn()
                            pend = []

                            # ---- projections ----
                            q_sb, k_sb, vh = [], [], []

                            def do_q():
                                if q_all:
                                    q_sb.extend(q_all_sb[2 * p:2 * p + 2])
                                    return
                                for j in range(2):
                                    hp = 2 * p + j
                                    pq = psum.tile([P, TOK], F32, tag="ps",
                                                   name="pq", bufs=3)
                                    for t in range(NB):
                                        nc.tensor.matmul(
                                            pq[:], cur[f"wq{j}"][:, t, :],
                                            q_src[t][:],
                                            start=(t == 0),
                                            stop=(t == NB - 1))
                                    qs = pool.tile([P, TOK], BF16, tag="qsb",
                                                   bufs=8)
                                    nc.vector.tensor_scalar_add(
                                        qs[:], pq[:], bq[:, hp:hp + 1])
                                    q_sb.append(qs)
                                    if dbg_stash is not None and \
                                            dbg_stash[0] == "q":
                                        dt_ = pool.tile([P, TOK], BF16,
                                                        tag=f"dq{hp}",
                                                        name=f"dq{hp}")
                                        nc.vector.tensor_copy(dt_[:], qs[:])
                                        dbg_stash[1].append(dt_)

                            def do_kv():
                                for j in range(2):
                                    ks = pool.tile([P, L], BF16, tag="ksb",
                                                   bufs=2)
                                    for half in range(2):
                                        sl = slice(half * 512,
                                                   (half + 1) * 512)
                                        pk = psum.tile([P, 512], F32,
                                                       tag="pkv", name="pk",
                                                       bufs=2)
                                        for t in range(NB):
                                            nc.tensor.matmul(
                                                pk[:], cur[f"wk{j}"][:, t, :],
                                                kvT[t][:, sl],
                                                start=(t == 0),
                                                stop=(t == NB - 1))
                                        nc.vector.tensor_copy(ks[:, sl],
                                                              pk[:])
                                    k_sb.append(ks)
                                    if dbg_stash is not None and \
                                            dbg_stash[0] == "k":
                                        hp = 2 * p + j
                                        dt_ = pool.tile([P, TOK], BF16,
                                                        tag=f"dk{hp}",
                                                        name=f"dk{hp}")
                                        nc.vector.tensor_copy(
                                            dt_[:], ks[:, 0:TOK])
                                        dbg_stash[1].append(dt_)
                                # V^T direct: vh[kb][key, head_in_pair, 0:64]
                                for kb in range(8):
                                    vt = pool.tile([P, 4, 65], BF16,
                                                   tag=f"vh{kb}",
                                                   name=f"vh{kb}", bufs=1)
                                    nc.gpsimd.memset(vt[:], 1.0)
                                    pvt = psum.tile([P, 256], F32, tag="pva",
                                                    name="pvt", bufs=3)
                                    for t in range(NB):
                                        nc.tensor.matmul(
                                            pvt[:],
                                            kvT[t][:, kb * P:(kb + 1) * P],
                                            cur["wv"][:, t, :],
                                            start=(t == 0),
                                            stop=(t == NB - 1))
                                    nc.vector.tensor_copy(
                                        vt[:, :, 0:64],
                                        pvt[:].rearrange("p (h c) -> p h c",
                                                         c=64))
                                    vh.append(vt)

                            if causal:
                                do_q()
                                do_kv()
                            else:
                                do_kv()
                                do_q()
                            if dbg_stash is not None and \
                                    dbg_stash[0] in ("q", "k"):
                                cur = nxt
                                continue

                            # ---- scores / softmax / AV ----
                            pts_hh = {}
                            pav_hh = {}
                            rec_hh = {}

                            def scores(hh):
                                j, lo = hh // 2, (hh % 2) * 64
                                hsl = slice(lo, lo + 64)
                                pts = []
                                for kb in range(8):
                                    smin = kb // 2 if causal else 0
                                    n = TOK - smin * P
                                    ns = SLOTS - smin
                                    ps = psum.tile([P, TOK], F32, tag="ps",
                                                   name="ps", bufs=3)
                                    nc.tensor.matmul(
                                        ps[:, 0:n],
                                        k_sb[j][hsl, kb * P:(kb + 1) * P],
                                        q_sb[j][hsl, smin * P:TOK])
                                    if causal:
                                        pe = pool.tile([P, TOK], BF16,
                                                       tag="pe", bufs=2)
                                        nc.scalar.activation(
                                            pe[:, 0:n], ps[:, 0:n], AF.Exp,
                                            scale=0.125)
                                        w0 = 256 * smin - 128 * kb + 128
                                        ebv = cur[f"eb{j}"][
                                            :, hh % 2,
                                            w0:w0 + ns * 256].rearrange(
                                            "p (s c) -> p s c",
                                            c=256)[:, :, 0:P]
                                        pt = pool.tile([P, TOK], BF16,
                                                       tag="pt", bufs=16)
                                        nc.vector.tensor_mul(
                                            pt[:, 0:n].rearrange(
                                                "p (s c) -> p s c", c=P),
                                            pe[:, 0:n].rearrange(
                                                "p (s c) -> p s c", c=P),
                                            ebv)
                                    else:
                                        pt = pool.tile([P, TOK], BF16,
                                                       tag="pe", bufs=16)
                                        nc.scalar.activation(
                                            pt[:, 0:n], ps[:, 0:n], AF.Exp,
                                            scale=0.125)
                                    pts.append(pt)
                                pts_hh[hh] = pts

                            def pav_f(hh):
                                pav = psum.tile([65, TOK], F32, tag="pva",
                                                name="pav", bufs=3)
                                for kb in range(8):
                                    smin = kb // 2 if causal else 0
                                    n = TOK - smin * P
                                    nc.tensor.matmul(
                                        pav[:, smin * P:TOK],
                                        vh[kb][:, hh, :],
                                        pts_hh[hh][kb][:, 0:n],
                                        start=(kb == 0), stop=(kb == 7))
                                # copy den to SBUF on DVE first: the custom
                                # DVE recip lacks cross-engine dep tracking,
                                # in-order DVE queue makes this safe
                                dsb = pool.tile([1, TOK], F32, tag="dens",
                                                bufs=2)
                                nc.vector.tensor_copy(dsb[:], pav[64:65, :])
                                rec = pool.tile([1, TOK], F32, tag="rec",
                                                bufs=2)
                                nc.vector.reciprocal_approx_fast(
                                    rec[:], dsb[:])
                                rcb = pool.tile([1, TOK], BF16, tag="recb",
                                                bufs=2)
                                nc.vector.tensor_copy(rcb[:], rec[:])
                                pav_hh[hh] = pav
                                rec_hh[hh] = rcb
                                if dbg_stash is not None and \
                                        dbg_stash[0] == "pav" and \
                                        p == 0 and hh == 0:
                                    dnum = pool.tile([64, TOK], F32,
                                                     tag="dnum")
                                    nc.vector.tensor_copy(dnum[:],
                                                          pav[0:64, :])
                                    dden = pool.tile([1, TOK], F32,
                                                     tag="dden")
                                    nc.vector.tensor_copy(dden[:],
                                                          pav[64:65, :])
                                    nc.sync.dma_start(
                                        out=d_out[0:64, 0:TOK],
                                        in_=dnum[:])
                                    nc.sync.dma_start(
                                        out=d_out[64:65, 0:TOK],
                                        in_=dden[:])
                                    nc.sync.dma_start(
                                        out=d_out[65:66, 0:TOK],
                                        in_=rec[:])

                            def norm_f(hh, hp, hsl):
                                def run():
                                    prr = psum.tile([64, TOK], F32, tag="ps",
                                                    name="prr", bufs=3)
                                    nc.tensor.matmul(
                                        prr[:], ones_row_bf[0:1, 0:64],
                                        rec_hh[hh][:])
                                    rrep = pool.tile([64, TOK], F32,
                                                     tag="rrep", bufs=2)
                                    nc.scalar.copy(out=rrep[:], in_=prr[:])
                                    nc.vector.tensor_mul(
                                        AO[hp][hsl, :], pav_hh[hh][0:64, :],
                                        rrep[:])
                                return run

                            scores(0)
                            scores(1)
                            pav_f(0)
                            scores(2)
                            pav_f(1)
                            norm_f(0, 2 * p, slice(0, 64))()
                            scores(3)
                            pav_f(2)
                            norm_f(1, 2 * p, slice(64, 128))()
                            pav_f(3)
                            pend = [norm_f(2, 2 * p + 1, slice(0, 64)),
                                    norm_f(3, 2 * p + 1, slice(64, 128))]
                            cur = nxt
                        for fn in pend:
                            fn()
                        if dbg_stash is not None:
                            attention.last_ao = AO
                            return {}

                        # ---- Wo + residual (+ LN head interleaved) ----
                        st = {}
                        wot = deque()

                        def fetch_wo(db):
                            t = wpool.tile([P, NB, P], BF16, tag="wot",
                                           bufs=2)
                            nc.sync.dma_start(out=t[:], in_=wov[db])
                            wot.append(t)

                        fetch_wo(0)
                        fetch_wo(1)
                        for db in range(NB):
                            if db + 2 < NB:
                                fetch_wo(db + 2)
                            w = wot.popleft()
                            po = psum.tile([P, TOK], F32, tag="ps", name="po",
                                           bufs=3)
                            for hp in range(NB):
                                nc.tensor.matmul(po[:], w[:, hp, :],
                                                 AO[hp][:],
                                                 start=(hp == 0),
                                                 stop=(hp == NB - 1))
                            rin = resid[db][:]
                            if rin.dtype == F32R:
                                rin = _f(rin)
                            nc.vector.scalar_tensor_tensor(
                                out_tiles[db][:], po[:], bo[:, db:db + 1],
                                rin, op0=AL.add, op1=AL.add)
                            ln_head(out_tiles, psum, "pkv", st, db)
                        attention.last_ao = AO
                        return st

                    with (
                        tc.tile_pool(name="x1mm", bufs=1) as x1mm,
                        tc.tile_pool(name="wp", bufs=1) as wpool,
                    ):
                        x1 = [x1mm.tile([P, TOK], BF16, tag=f"x1_{d}",
                                        name=f"x1_{d}") for d in range(NB)]
                        mm_t = x1mm.tile([P, NB, L], BF16, tag="mmall")
                        mm = [mm_t[:, t, :] for t in range(NB)]

                        # ------------- self-attention -------------
                        with (
                            tc.tile_pool(name="sa", bufs=1) as sa_pool,
                            tc.tile_pool(name="sa_ps", bufs=1,
                                         space="PSUM") as sa_psum,
                        ):
                            xo_t = sa_pool.tile([P, NB, TOK], BF16,
                                                tag="xoall")
                            nc.sync.dma_start(out=xo_t[:], in_=d_xo[:, :, :])
                            xo = [xo_t[:, t, :] for t in range(NB)]
                            xf_t = sa_pool.tile([P, NB, L], BF16,
                                                tag="xfall")
                            xf = [xf_t[:, t, :] for t in range(NB)]

                            def early():
                                nc.sync.dma_start(out=xf_t[:],
                                                  in_=d_xf[:, :, :])
                                nc.sync.dma_start(out=mm_t[:],
                                                  in_=d_mem[:, :, :])

                            stash = ([dbg, []]
                                     if dbg in ("q", "k", "pav") else None)
                            st = attention(xo, xf, "sa", True, x1, xo,
                                           sa_pool, sa_psum, early_dmas=early,
                                           dbg_stash=stash, q_all=True)
                            if dbg not in ("x1pre", "q", "k", "pav"):
                                ln_tail(x1, "1", sa_psum, "ps", "pva", st,
                                        r_bufs=3)
                            if dbg in ("x1pre", "x1", "ao", "q", "k"):
                                dsrc = (attention.last_ao if dbg == "ao"
                                        else stash[1] if stash else x1)
                                identb = sa_pool.tile([P, P], BF16,
                                                      tag="identb")
                                nc.scalar.copy(out=identb[:], in_=identf[:])
                                osb = [sa_pool.tile([P, D], F32,
                                                    tag=f"dos{s}",
                                                    name=f"dos{s}")
                                       for s in range(SLOTS)]
                                for db in range(NB):
                                    for s in range(SLOTS):
                                        pd = sa_psum.tile([P, P], BF16,
                                                          tag="ps", bufs=3)
                                        nc.tensor.matmul(
                                            pd[:],
                                            dsrc[db][:, s * P:(s + 1) * P],
                                            identb[:], is_transpose=True)
                                        nc.vector.tensor_copy(
                                            osb[s][:, db * P:(db + 1) * P],
                                            pd[:])
                                for s in range(SLOTS):
                                    nc.sync.dma_start(
                                        out=d_out[s * P:(s + 1) * P, :],
                                        in_=osb[s][:])

                        # ------------- cross-attention -------------
                        if dbg is None:
                            with (
                                tc.tile_pool(name="ca", bufs=1) as ca_pool,
                                tc.tile_pool(name="ca_ps", bufs=1,
                                             space="PSUM") as ca_psum,
                            ):
                                st = attention(x1, mm, "ca", False, x2, x1,
                                               ca_pool, ca_psum)
                                ln_tail(x2, "2", ca_psum, "ps", "pva", st,
                                        r_bufs=3)

                    # ---------------- FFN ----------------
                    if dbg is not None:
                        continue
                    with (
                        tc.tile_pool(name="ff", bufs=1) as ff_pool,
                        tc.tile_pool(name="ff_ps", bufs=1,
                                     space="PSUM") as ff_psum,
                    ):
                        w1q = deque()

                        def fetch_w1(ff):
                            t = ff_pool.tile([P, NB, P], BF16, tag="w1t",
                                             bufs=3)
                            nc.sync.dma_start(out=t[:], in_=d_fc1[ff])
                            w1q.append(t)

                        fetch_w1(0)
                        fetch_w1(1)
                        ht = []
                        for ff in range(NF):
                            if ff + 2 < NF:
                                fetch_w1(ff + 2)
                            w1 = w1q.popleft()
                            pf = ff_psum.tile([P, TOK], F32, tag="pf",
                                              name="pf", bufs=2)
                            for t in range(NB):
                                nc.tensor.matmul(pf[:], w1[:, t, :],
                                                 x2[t][:],
                                                 start=(t == 0),
                                                 stop=(t == NB - 1))
                            h = ff_pool.tile([P, TOK], BF16, tag=f"ht{ff}",
                                             name=f"ht{ff}")
                            nc.scalar.activation(h[:], pf[:], AF.Relu,
                                                 bias=fc1b[:, ff:ff + 1],
                                                 scale=1.0)
                            ht.append(h)
                        x3 = [ff_pool.tile([P, TOK], F32R, tag=f"x3_{d}",
                                           name=f"x3_{d}")
                              for d in range(NB)]
                        w2q = deque()

                        def fetch_w2(db):
                            t = ff_pool.tile([P, NF, P], BF16, tag="w2t",
                                             bufs=2)
                            nc.sync.dma_start(out=t[:], in_=d_fc2[db])
                            w2q.append(t)

                        fetch_w2(0)
                        fetch_w2(1)
                        st = {}
                        for db in range(NB):
                            if db + 2 < NB:
                                fetch_w2(db + 2)
                            w2 = w2q.popleft()
                            pf2 = ff_psum.tile([P, TOK], F32, tag="pf2",
                                               name="pf2", bufs=2)
                            for t in range(NF):
                                nc.tensor.matmul(pf2[:], w2[:, t, :],
                                                 ht[t][:],
                                                 start=(t == 0),
                                                 stop=(t == NF - 1))
                            nc.vector.scalar_tensor_tensor(
                                x3[db][:], pf2[:], fc2b[:, db:db + 1],
                                x2[db][:], op0=AL.add, op1=AL.add)
                            ln_head(x3, ff_psum, "pf", st, db, bf=False)

                        outsb = [ff_pool.tile([P, D], F32, tag=f"os{s}",
                                              name=f"os{s}")
                                 for s in range(SLOTS)]

                        def post_blk(db):
                            for s in range(SLOTS):
                                ptr = ff_psum.tile([P, P], F32, tag="ptr",
                                                   name="ptr", bufs=2)
                                nc.tensor.matmul(
                                    _r(ptr[:]),
                                    _r(x3[db][:, s * P:(s + 1) * P]),
                                    _r(ident[:]), is_transpose=True)
                                nc.vector.tensor_copy(
                                    outsb[s][:, db * P:(db + 1) * P],
                                    ptr[:])

                        ln_tail(x3, "3", ff_psum, "pf2", "ptr", st,
                                post_blk=post_blk, bf=False)
                        for s in range(SLOTS):
                            nc.sync.dma_start(
                                out=d_out[s * P:(s + 1) * P, :],
                                in_=outsb[s][:])

    nc.finalize()
    return nc


@functools.lru_cache(maxsize=4)
def _get_nc(reps=1, dbg=None):
    return _build_nc(reps, dbg)


def _rel_bucket_np(v):
    """T5 causal bucket for relative distance v = q - k (>= 0)."""
    n = np.maximum(v, 0)
    max_exact = NUM_BUCKETS // 2
    nf = np.maximum(n.astype(np.float32), 1.0)
    val_large = max_exact + (
        np.log(nf / max_exact) / math.log(MAX_DISTANCE / max_exact)
        * (NUM_BUCKETS - max_exact)
    ).astype(np.int32)
    val_large = np.minimum(val_large, NUM_BUCKETS - 1)
    return np.where(n < max_exact, n, val_large).astype(np.int32)


def _build_eb(rel_emb, g):
    """EB[h, i, w] = exp(band_h((w - 128 + 128 g) - i)); 0 where q < k."""
    v = (np.arange(1024)[None, :] - 128 + 128 * g) - np.arange(P)[:, None]
    bucket = _rel_bucket_np(v)                      # [128, 1024]
    band = rel_emb[bucket]                          # [128, 1024, 16]
    band = np.transpose(band, (2, 0, 1)).astype(np.float64)  # [16, 128, 1024]
    eb = np.exp(band)
    eb[:, v < 0] = 0.0
    out = np.zeros((H, P, 1280), dtype=np.float32)
    out[:, :, :1024] = eb
    return out.astype(ml_dtypes.bfloat16)


def _rearr_bias(b):
    return np.ascontiguousarray(b.reshape(-1, P).T, dtype=np.float32)


def _tile4(w, dt=ml_dtypes.bfloat16):
    kb, mb = w.shape[0] // P, w.shape[1] // P
    return np.ascontiguousarray(
        w.reshape(kb, P, mb, P).transpose(2, 1, 0, 3)).astype(dt)


def _tile_v2(w):
    """[K, M] -> [M//256, K//128, 128(k), 256(m)] for the V^T moving op."""
    kb, mb2 = w.shape[0] // P, w.shape[1] // 256
    r = w.reshape(kb, P, mb2, 256).transpose(2, 1, 0, 3)
    # want [pair, P(k_in), kb, 256] with partition dim = k_in
    return np.ascontiguousarray(r).astype(ml_dtypes.bfloat16)


def _gb_stack(g, b):
    return np.ascontiguousarray(
        np.stack([g.reshape(NB, P), b.reshape(NB, P)], axis=0)
    ).astype(ml_dtypes.bfloat16)


def _make_in_maps(inp):
    x = np.asarray(inp["x"], np.float32)
    mem = np.asarray(inp["mem"], np.float32)
    rel_emb = np.asarray(inp["rel_emb"], np.float32)

    shared = {}
    for k in ("sa_wq", "sa_wk", "sa_wo", "ca_wq", "ca_wk", "ca_wo",
              "fc1_w", "fc2_w"):
        shared[k] = _tile4(np.asarray(inp[k]))
    shared["sa_wv2"] = _tile_v2(np.asarray(inp["sa_wv"]))
    shared["ca_wv2"] = _tile_v2(np.asarray(inp["ca_wv"]))
    cols = []
    for pre in ("sa", "ca"):
        bo = np.asarray(inp[f"{pre}_bo"]) + \
            np.asarray(inp[f"{pre}_bv"]) @ np.asarray(inp[f"{pre}_wo"])
        cols.append(_rearr_bias(np.asarray(inp[f"{pre}_bq"])))
        cols.append(_rearr_bias(bo))
    # reorder: sa_bq, sa_bo, ca_bq, ca_bo
    cols = [cols[0], cols[1], cols[2], cols[3],
            _rearr_bias(np.asarray(inp["fc1_b"])),
            _rearr_bias(np.asarray(inp["fc2_b"])),
            _rearr_bias(np.asarray(inp["ln1_g"])),
            _rearr_bias(np.asarray(inp["ln2_g"])),
            _rearr_bias(np.asarray(inp["ln3_g"]))]
    shared["bias_all"] = np.ascontiguousarray(
        np.concatenate(cols, axis=1), np.float32)
    gbs = []
    for i in ("1", "2", "3"):
        gbs.append(np.asarray(inp[f"ln{i}_g"]).reshape(NB, P))
        gbs.append(np.asarray(inp[f"ln{i}_b"]).reshape(NB, P))
    shared["gb_all"] = np.ascontiguousarray(
        np.stack(gbs, axis=0)[None]).astype(ml_dtypes.bfloat16)
    eb = [_build_eb(rel_emb, g) for g in range(2)]
    # [H, P, 1280] -> [NB hp, P, 2, 1280]
    eb = [np.ascontiguousarray(
        e.reshape(NB, 2, P, 1280).transpose(0, 2, 1, 3)) for e in eb]

    def _blk(a):
        # [D, N] -> [P, NB, N]
        return np.ascontiguousarray(
            a.reshape(NB, P, a.shape[1]).transpose(1, 0, 2))

    in_maps = []
    for c in range(8):
        b, g = c // 2, c % 2
        rows = np.concatenate(
            [x[b, (2 * s + g) * P:(2 * s + g + 1) * P] for s in range(SLOTS)])
        m = dict(shared)
        m["xoT"] = _blk(np.ascontiguousarray(rows.T)).astype(
            ml_dtypes.bfloat16)
        m["xfT"] = _blk(np.ascontiguousarray(x[b].T)).astype(
            ml_dtypes.bfloat16)
        m["memT"] = _blk(np.ascontiguousarray(mem[b].T)).astype(
            ml_dtypes.bfloat16)
        m["eb"] = eb[g]
        in_maps.append(m)
    return in_maps


def kernel(x, mem, tgt_mask, mem_mask,
           sa_wq, sa_bq, sa_wk, sa_bk, sa_wv, sa_bv, sa_wo, sa_bo, rel_emb,
           ca_wq, ca_bq, ca_wk, ca_bk, ca_wv, ca_bv, ca_wo, ca_bo,
           fc1_w, fc1_b, fc2_w, fc2_b,
           ln1_g, ln1_b, ln2_g, ln2_b, ln3_g, ln3_b, _trace=False):
    nc = _get_nc()
    in_maps = _make_in_maps(dict(
        x=x, mem=mem, rel_emb=rel_emb,
        sa_wq=sa_wq, sa_wk=sa_wk, sa_wv=sa_wv, sa_wo=sa_wo,
        sa_bq=sa_bq, sa_bk=sa_bk, sa_bv=sa_bv, sa_bo=sa_bo,
        ca_wq=ca_wq, ca_wk=ca_wk, ca_wv=ca_wv, ca_wo=ca_wo,
        ca_bq=ca_bq, ca_bk=ca_bk, ca_bv=ca_bv, ca_bo=ca_bo,
        fc1_w=fc1_w, fc1_b=fc1_b, fc2_w=fc2_w, fc2_b=fc2_b,
        ln1_g=ln1_g, ln1_b=ln1_b, ln2_g=ln2_g, ln2_b=ln2_b,
        ln3_g=ln3_g, ln3_b=ln3_b))

    res = run_bass_kernel_spmd(nc, in_maps, list(range(8)), trace=_trace)
    out = np.empty((B, L, D), np.float32)
    for c in range(8):
        b, g = c // 2, c % 2
        oc = res.results[c]["out_own"]
        for s in range(SLOTS):
            out[b, (2 * s + g) * P:(2 * s + g + 1) * P] = \
                oc[s * P:(s + 1) * P]
    kernel.last_exec_time_ns = res.exec_time_ns
    return out


# revision 78
# speedup vs baseline: 1178.3234x; 1.0228x over previous
"""Trainium2 Bass kernel for a T5-style decoder layer (self-attn with causal
rel-pos bias, cross-attn, FFN, 3 post-LNs).

Sharding: 8 cores = (batch b in 0..3) x (parity g in 0..1). Core (b, g) owns
query blocks {g, 2+g, 4+g, 6+g} (128 rows each) of batch b and computes the
full layer for those 512 rows. K/V work is duplicated across the pair; no
cross-core collectives are needed. Causal score work is padded to a uniform
(2,4,6,8) key-block pattern so one SPMD program serves all cores; padded
blocks are killed by the exp-band (host bakes exp(bias+mask), 0 where
masked).

v2 design notes (vs v0 baseline):
- all matmul *moving* operands are bf16 (cycles/row = 1.0 at any free size);
  residual stream x1/x2/x3 stays f32r for accuracy.
- rel-pos bias enters as EB = exp(band) bf16, applied by a DVE multiply on
  the exp'd scores (2x DVE mode) instead of an identity-matmul accumulate.
- K/V biases dropped: bk shifts every score of a query equally (softmax
  invariant); bv contributes bv@wo to the output (folded into bo on host).
- V^T is built directly (out[key, vdim] = sum_d x[d,key] * wv[d,vdim]) per
  hp-pair with free=256, killing the per-head PE transposes.
- LN: out = x*(g (x) rsd) + R with rank-2 R = g (x) nmr + b (x) 1 computed by
  one PE matmul per block; mean/var reductions interleaved into the Wo / fc2
  loops; reciprocal_approx_fast everywhere.
- weight/EB/mem prefetch via rotating pools so the PE never waits on DMA at
  phase boundaries.
"""

import functools
import math
from collections import deque

import ml_dtypes
import numpy as np

import concourse.bass as bass
import concourse.bacc as bacc
import concourse.mybir as mybir
import concourse.tile as tile
from concourse.bass_utils import run_bass_kernel_spmd
from concourse.masks import make_identity

F32 = mybir.dt.float32
F32R = mybir.dt.float32r
BF16 = mybir.dt.bfloat16
AL = mybir.AluOpType
AF = mybir.ActivationFunctionType

B, L, D, H, DK, DFF = 4, 1024, 1024, 16, 64, 4096
P = 128
NB = D // P            # 8 d_model blocks
NF = DFF // P          # 32 d_ff blocks
TOK = 512              # tokens owned per core
SLOTS = 4              # query blocks of 128 per core
NPAIR = 4              # hp pairs (each pair = 4 heads = 256 qkv dims)
NUM_BUCKETS, MAX_DISTANCE = 32, 128
EPS = 1e-5


def _r(x):
    return x.bitcast(F32R)


def _f(x):
    return x.bitcast(F32)


def _build_nc(reps=1, dbg=None):
    nc = bacc.Bacc(trn_type="TRN2")

    def inp(name, shape, dt=F32):
        return nc.declare_dram_parameter(name, list(shape), dt, isOutput=False)

    d_xo = inp("xoT", (P, NB, TOK), BF16)   # own tokens: Q moving + residual
    d_xf = inp("xfT", (P, NB, L), BF16)     # all tokens, self K/V source
    d_mem = inp("memT", (P, NB, L), BF16)   # memory, cross K/V source
    d_eb = inp("eb", (NB, P, 2, 1280), BF16)  # exp(band), paired per hp
    d_ball = inp("bias_all", (P, 96))        # packed per-partition consts
    d_gball = inp("gb_all", (1, 6, NB, P), BF16)  # LN g/b rows
    dw = {}
    for pre in ("sa", "ca"):
        for nm in ("wq", "wk", "wo"):
            dw[f"{pre}_{nm}"] = inp(f"{pre}_{nm}", (NB, P, NB, P), BF16)
        dw[f"{pre}_wv2"] = inp(f"{pre}_wv2", (NPAIR, P, NB, 256), BF16)
    d_fc1 = inp("fc1_w", (NF, P, NB, P), BF16)
    d_fc2 = inp("fc2_w", (NB, P, NF, P), BF16)
    d_out = nc.declare_dram_parameter("out_own", [TOK, D], F32, isOutput=True)

    with (
        nc.allow_low_precision(reason="bf16 matmul streams; tol 2e-2"),
        tile.TileContext(nc) as tc,
    ):
        with tc.tile_pool(name="persist", bufs=1) as pers:
            identf = pers.tile([P, P], F32, tag="identf")
            make_identity(nc, identf[:])
            ident = pers.tile([P, P], F32R, tag="ident")
            nc.scalar.copy(out=ident[:], in_=identf[:])
            ones_col = pers.tile([P, 1], F32R, tag="ones")
            nc.gpsimd.memset(_f(ones_col[:]), 1.0)
            ones_bf = pers.tile([P, 1], BF16, tag="onesb")
            nc.gpsimd.memset(ones_bf[:], 1.0)
            ones_row = pers.tile([1, P], F32R, tag="onesr")
            nc.gpsimd.memset(_f(ones_row[:]), 1.0)
            ones_row_bf = pers.tile([1, P], BF16, tag="onesrb")
            nc.gpsimd.memset(ones_row_bf[:], 1.0)
            eps_t = pers.tile([1, 1], F32, tag="epsc")
            nc.gpsimd.memset(eps_t[:], EPS)

            ball = pers.tile([P, 96], F32, tag="ball")
            nc.sync.dma_start(out=ball[:], in_=d_ball[:, :])
            gball = pers.tile([1, 6, NB, P], BF16, tag="gball")
            nc.sync.dma_start(out=gball[:], in_=d_gball[:, :, :, :])
            bias_sb = {
                "sa_bq": ball[:, 0:8], "sa_bo": ball[:, 8:16],
                "ca_bq": ball[:, 16:24], "ca_bo": ball[:, 24:32],
            }
            fc1b = ball[:, 32:64]
            fc2b = ball[:, 64:72]
            ln_sb = {}
            for ii, i in enumerate(("1", "2", "3")):
                ln_sb[f"g{i}"] = ball[:, 72 + 8 * ii:80 + 8 * ii]
                ln_sb[f"gr{i}"] = gball[:, 2 * ii, :, :]
                ln_sb[f"br{i}"] = gball[:, 2 * ii + 1, :, :]

            for _rep in range(reps):
                with tc.tile_pool(name="outer", bufs=1) as outer:
                    x2 = [outer.tile([P, TOK], BF16, tag=f"x2_{d}",
                                     name=f"x2_{d}") for d in range(NB)]
                    nmr_t = outer.tile([1, TOK], BF16, tag="nmr")
                    ones_tok = outer.tile([1, TOK], BF16, tag="onestok")
                    nc.gpsimd.memset(ones_tok[:], 1.0)
                    rsd_t = outer.tile([1, TOK], F32, tag="rsd")
                    rsd_bf = outer.tile([1, TOK], BF16, tag="rsdb")
                    rsdr = outer.tile([P, TOK], BF16, tag="rsdr")

                    # ---- layernorm helpers (head interleaved with caller
                    # loop via ln_head(db); tail emits per-block outputs) ----
                    def ln_head(src, psum, acc_tag, state, db, bf=True):
                        ones = ones_bf if bf else ones_col
                        if db == 0:
                            state["pm"] = psum.tile([1, TOK], F32, tag=acc_tag,
                                                    name="pm", bufs=2)
                            state["pv2"] = psum.tile([1, TOK], F32,
                                                     tag=acc_tag, name="pv2",
                                                     bufs=2)
                        nc.tensor.matmul(state["pm"][:], ones[:],
                                         src[db][:],
                                         start=(db == 0), stop=(db == NB - 1))
                        if bf:
                            sq = outer.tile([P, TOK], BF16, tag="sqb", bufs=2)
                            nc.scalar.square(sq[:], src[db][:])
                        else:
                            sq = outer.tile([P, TOK], F32R, tag="sq", bufs=2)
                            nc.scalar.square(sq[:], _f(src[db][:]))
                        nc.tensor.matmul(state["pv2"][:], ones[:],
                                         sq[:],
                                         start=(db == 0), stop=(db == NB - 1))

                    def ln_tail(src, gi, psum, r_tag, bc_tag, state,
                                post_blk=None, r_bufs=2, bf=True):
                        g_ap = ln_sb[f"g{gi}"]
                        gr_ap = ln_sb[f"gr{gi}"]
                        br_ap = ln_sb[f"br{gi}"]
                        pm, pv2 = state["pm"], state["pv2"]
                        mu = outer.tile([1, TOK], F32, tag="mu")
                        nc.vector.tensor_scalar_mul(mu[:], pm[:], 1.0 / D)
                        musq = outer.tile([1, TOK], F32, tag="musq")
                        nc.vector.tensor_mul(musq[:], mu[:], mu[:])
                        var = outer.tile([1, TOK], F32, tag="var")
                        nc.vector.scalar_tensor_tensor(
                            var[:], pv2[:], 1.0 / D, musq[:],
                            op0=AL.mult, op1=AL.subtract)
                        std = outer.tile([1, TOK], F32, tag="std")
                        nc.scalar.activation(std[:], var[:], AF.Sqrt,
                                             bias=eps_t[:])
                        std_c = outer.tile([1, TOK], F32, tag="stdc")
                        nc.vector.tensor_copy(std_c[:], std[:])
                        nc.vector.reciprocal_approx_fast(rsd_t[:], std_c[:])
                        nc.vector.tensor_copy(rsd_bf[:], rsd_t[:])
                        nc.vector.scalar_tensor_tensor(
                            nmr_t[:], mu[:], -1.0, rsd_t[:],
                            op0=AL.mult, op1=AL.mult)
                        pbc = psum.tile([P, TOK], F32, tag=bc_tag, name="pbc",
                                        bufs=r_bufs)
                        nc.tensor.matmul(pbc[:], ones_row_bf[:],
                                         rsd_bf[:])
                        nc.scalar.copy(out=rsdr[:], in_=pbc[:])
                        for db in range(NB):
                            pr = psum.tile([P, TOK], F32, tag=r_tag, name="pr",
                                           bufs=r_bufs)
                            nc.tensor.matmul(pr[:], gr_ap[:, db, :],
                                             nmr_t[:],
                                             start=True, stop=False)
                            nc.tensor.matmul(pr[:], br_ap[:, db, :],
                                             ones_tok[:],
                                             start=False, stop=True)
                            if bf:
                                t1 = outer.tile([P, TOK], BF16, tag="t1b",
                                                bufs=2)
                                nc.vector.scalar_tensor_tensor(
                                    t1[:], src[db][:], g_ap[:, db:db + 1],
                                    rsdr[:], op0=AL.mult, op1=AL.mult)
                            else:
                                t1 = outer.tile([P, TOK], F32, tag="t1",
                                                bufs=2)
                                nc.vector.scalar_tensor_tensor(
                                    t1[:], _f(src[db][:]),
                                    g_ap[:, db:db + 1],
                                    rsdr[:], op0=AL.mult, op1=AL.mult)
                            nc.vector.tensor_add(src[db][:], t1[:], pr[:])
                            if post_blk is not None:
                                post_blk(db)

                    # ================= attention =================
                    def attention(q_src, kvT, pre, causal, out_tiles, resid,
                                  pool, psum, early_dmas=None, dbg_stash=None,
                                  q_all=False):
                        wqv, wkv = dw[f"{pre}_wq"], dw[f"{pre}_wk"]
                        wvv, wov = dw[f"{pre}_wv2"], dw[f"{pre}_wo"]
                        bq, bo = bias_sb[f"{pre}_bq"], bias_sb[f"{pre}_bo"]
                        AO = [pool.tile([P, TOK], BF16, tag=f"ao{hp}",
                                        name=f"ao{hp}", bufs=1)
                              for hp in range(NB)]

                        def fetch(p, with_q=True):
                            ws = {}
                            for j in range(2):
                                hp = 2 * p + j
                                if with_q:
                                    wq_t = wpool.tile([P, NB, P], BF16,
                                                      tag="wqt", bufs=8)
                                    nc.sync.dma_start(out=wq_t[:],
                                                      in_=wqv[hp])
                                    ws[f"wq{j}"] = wq_t
                                wk_t = wpool.tile([P, NB, P], BF16, tag="wkt",
                                                  bufs=4)
                                nc.sync.dma_start(out=wk_t[:], in_=wkv[hp])
                                ws[f"wk{j}"] = wk_t
                                if causal:
                                    eb = pool.tile([P, 2, 1280], BF16,
                                                   tag="ebt", bufs=2)
                                    nc.sync.dma_start(out=eb[:],
                                                      in_=d_eb[hp])
                                    ws[f"eb{j}"] = eb
                            wv_t = wpool.tile([P, NB, 256], BF16, tag="wvt",
                                              bufs=2)
                            nc.sync.dma_start(out=wv_t[:], in_=wvv[p])
                            ws["wv"] = wv_t
                            return ws

                        q_all_sb = []
                        if q_all:
                            wq_ts = []
                            for hp in range(NB):
                                wq_t = wpool.tile([P, NB, P], BF16,
                                                  tag="wqt", bufs=8)
                                nc.sync.dma_start(out=wq_t[:], in_=wqv[hp])
                                wq_ts.append(wq_t)
                            if early_dmas is not None:
                                early_dmas()
                            cur = fetch(0, with_q=False)
                            for hp in range(NB):
                                pq = psum.tile([P, TOK], F32, tag="ps",
                                               name="pq", bufs=3)
                                for t in range(NB):
                                    nc.tensor.matmul(
                                        pq[:], wq_ts[hp][:, t, :],
                                        q_src[t][:],
                                        start=(t == 0), stop=(t == NB - 1))
                                qs = pool.tile([P, TOK], BF16, tag="qsb",
                                               name="qsf", bufs=8)
                                nc.vector.tensor_scalar_add(
                                    qs[:], pq[:], bq[:, hp:hp + 1])
                                q_all_sb.append(qs)
                        else:
                            cur = fetch(0)
                            if early_dmas is not None:
                                early_dmas()

                        wot = deque()

                        def fetch_wo(db):
                            t = wpool.tile([P, NB, P], BF16, tag="wot",
                                           bufs=2)
                            nc.sync.dma_start(out=t[:], in_=wov[db])
                            wot.append(t)

                        pend = []   # deferred normalize from previous pair
                        for p in range(NPAIR):
                            if p == NPAIR - 1:
                                fetch_wo(0)
                                fetch_wo(1)
                            nxt = (fetch(p + 1, with_q=not q_all)
                                   if p + 1 < NPAIR else None)
                            # flush previous pair's normalize (recips done)
                            for fn in pend:
                                fn()
                            pend = []

                            # ---- projections ----
                            q_sb, k_sb, vh = [], [], []

                            def do_q():
                                if q_all:
                                    q_sb.extend(q_all_sb[2 * p:2 * p + 2])
                                    return
                                for j in range(2):
                                    hp = 2 * p + j
                                    pq = psum.tile([P, TOK], F32, tag="ps",
                                                   name="pq", bufs=3)
                                    for t in range(NB):
                                        nc.tensor.matmul(
                                            pq[:], cur[f"wq{j}"][:, t, :],
                                            q_src[t][:],
                                            start=(t == 0),
                                            stop=(t == NB - 1))
                                    qs = pool.tile([P, TOK], BF16, tag="qsb",
                                                   bufs=8)
                                    nc.vector.tensor_scalar_add(
                                        qs[:], pq[:], bq[:, hp:hp + 1])
                                    q_sb.append(qs)
                                    if dbg_stash is not None and \
                                            dbg_stash[0] == "q":
                                        dt_ = pool.tile([P, TOK], BF16,
                                                        tag=f"dq{hp}",
                                                        name=f"dq{hp}")
                                        nc.vector.tensor_copy(dt_[:], qs[:])
                                        dbg_stash[1].append(dt_)

                            def do_kv():
                                for j in range(2):
                                    ks = pool.tile([P, L], BF16, tag="ksb",
                                                   bufs=2)
                                    for half in range(2):
                                        sl = slice(half * 512,
                                                   (half + 1) * 512)
                                        pk = psum.tile([P, 512], F32,
                                                       tag="pkv", name="pk",
                                                       bufs=2)
                                        for t in range(NB):
                                            nc.tensor.matmul(
                                                pk[:], cur[f"wk{j}"][:, t, :],
                                                kvT[t][:, sl],
                                                start=(t == 0),
                                                stop=(t == NB - 1))
                                        nc.vector.tensor_copy(ks[:, sl],
                                                              pk[:])
                                    k_sb.append(ks)
                                    if dbg_stash is not None and \
                                            dbg_stash[0] == "k":
                                        hp = 2 * p + j
                                        dt_ = pool.tile([P, TOK], BF16,
                                                        tag=f"dk{hp}",
                                                        name=f"dk{hp}")
                                        nc.vector.tensor_copy(
                                            dt_[:], ks[:, 0:TOK])
                                        dbg_stash[1].append(dt_)
                                # V^T direct: vh[kb][key, head_in_pair, 0:64]
                                for kb in range(8):
                                    vt = pool.tile([P, 4, 65], BF16,
                                                   tag=f"vh{kb}",
                                                   name=f"vh{kb}", bufs=1)
                                    nc.gpsimd.memset(vt[:], 1.0)
                                    pvt = psum.tile([P, 256], F32, tag="pva",
                                                    name="pvt", bufs=3)
                                    for t in range(NB):
                                        nc.tensor.matmul(
                                            pvt[:],
                                            kvT[t][:, kb * P:(kb + 1) * P],
                                            cur["wv"][:, t, :],
                                            start=(t == 0),
                                            stop=(t == NB - 1))
                                    nc.vector.tensor_copy(
                                        vt[:, :, 0:64],
                                        pvt[:].rearrange("p (h c) -> p h c",
                                                         c=64))
                                    vh.append(vt)

                            if causal:
                                do_q()
                                do_kv()
                            else:
                                do_kv()
                                do_q()
                            if dbg_stash is not None and \
                                    dbg_stash[0] in ("q", "k"):
                                cur = nxt
                                continue

                            # ---- scores / softmax / AV ----
                            pts_hh = {}
                            pav_hh = {}
                            rec_hh = {}

                            def scores(hh):
                                j, lo = hh // 2, (hh % 2) * 64
                                hsl = slice(lo, lo + 64)
                                pts = []
                                for kb in range(8):
                                    smin = kb // 2 if causal else 0
                                    n = TOK - smin * P
                                    ns = SLOTS - smin
                                    ps = psum.tile([P, TOK], F32, tag="ps",
                                                   name="ps", bufs=3)
                                    nc.tensor.matmul(
                                        ps[:, 0:n],
                                        k_sb[j][hsl, kb * P:(kb + 1) * P],
                                        q_sb[j][hsl, smin * P:TOK])
                                    if causal:
                                        pe = pool.tile([P, TOK], BF16,
                                                       tag="pe", bufs=2)
                                        nc.scalar.activation(
                                            pe[:, 0:n], ps[:, 0:n], AF.Exp,
                                            scale=0.125)
                                        w0 = 256 * smin - 128 * kb + 128
                                        ebv = cur[f"eb{j}"][
                                            :, hh % 2,
                                            w0:w0 + ns * 256].rearrange(
                                            "p (s c) -> p s c",
                                            c=256)[:, :, 0:P]
                                        pt = pool.tile([P, TOK], BF16,
                                                       tag="pt", bufs=16)
                                        nc.vector.tensor_mul(
                                            pt[:, 0:n].rearrange(
                                                "p (s c) -> p s c", c=P),
                                            pe[:, 0:n].rearrange(
                                                "p (s c) -> p s c", c=P),
                                            ebv)
                                    else:
                                        pt = pool.tile([P, TOK], BF16,
                                                       tag="pe", bufs=16)
                                        nc.scalar.activation(
                                            pt[:, 0:n], ps[:, 0:n], AF.Exp,
                                            scale=0.125)
                                    pts.append(pt)
                                pts_hh[hh] = pts

                            def pav_f(hh):
                                pav = psum.tile([65, TOK], F32, tag="pva",
                                                name="pav", bufs=3)
                                for kb in range(8):
                                    smin = kb // 2 if causal else 0
                                    n = TOK - smin * P
                                    nc.tensor.matmul(
                                        pav[:, smin * P:TOK],
                                        vh[kb][:, hh, :],
                                        pts_hh[hh][kb][:, 0:n],
                                        start=(kb == 0), stop=(kb == 7))
                                # copy den to SBUF on DVE first: the custom
                                # DVE recip lacks cross-engine dep tracking,
                                # in-order DVE queue makes this safe
                                dsb = pool.tile([1, TOK], F32, tag="dens",
                                                bufs=2)
                                nc.vector.tensor_copy(dsb[:], pav[64:65, :])
                                rec = pool.tile([1, TOK], F32, tag="rec",
                                                bufs=2)
                                nc.vector.reciprocal_approx_fast(
                                    rec[:], dsb[:])
                                rcb = pool.tile([1, TOK], BF16, tag="recb",
                                                bufs=2)
                                nc.vector.tensor_copy(rcb[:], rec[:])
                                pav_hh[hh] = pav
                                rec_hh[hh] = rcb
                                if dbg_stash is not None and \
                                        dbg_stash[0] == "pav" and \
                                        p == 0 and hh == 0:
                                    dnum = pool.tile([64, TOK], F32,
                                                     tag="dnum")
                                    nc.vector.tensor_copy(dnum[:],
                                                          pav[0:64, :])
                                    dden = pool.tile([1, TOK], F32,
                                                     tag="dden")
                                    nc.vector.tensor_copy(dden[:],
                                                          pav[64:65, :])
                                    nc.sync.dma_start(
                                        out=d_out[0:64, 0:TOK],
                                        in_=dnum[:])
                                    nc.sync.dma_start(
                                        out=d_out[64:65, 0:TOK],
                                        in_=dden[:])
                                    nc.sync.dma_start(
                                        out=d_out[65:66, 0:TOK],
                                        in_=rec[:])

                            def norm_f(hh, hp, hsl):
                                def run():
                                    prr = psum.tile([64, TOK], F32, tag="ps",
                                                    name="prr", bufs=3)
                                    nc.tensor.matmul(
                                        prr[:], ones_row_bf[0:1, 0:64],
                                        rec_hh[hh][:])
                                    rrep = pool.tile([64, TOK], F32,
                                                     tag="rrep", bufs=2)
                                    nc.scalar.copy(out=rrep[:], in_=prr[:])
                                    nc.vector.tensor_mul(
                                        AO[hp][hsl, :], pav_hh[hh][0:64, :],
                                        rrep[:])
                                return run

                            scores(0)
                            scores(1)
                            pav_f(0)
                            scores(2)
                            pav_f(1)
                            norm_f(0, 2 * p, slice(0, 64))()
                            scores(3)
                            pav_f(2)
                            norm_f(1, 2 * p, slice(64, 128))()
                            pav_f(3)
                            pend = [norm_f(2, 2 * p + 1, slice(0, 64)),
                                    norm_f(3, 2 * p + 1, slice(64, 128))]
                            cur = nxt
                        for fn in pend:
                            fn()
                        if dbg_stash is not None:
                            attention.last_ao = AO
                            return {}

                        # ---- Wo + residual (+ LN head interleaved) ----
                        st = {}
                        for db in range(NB):
                            if db + 2 < NB:
                                fetch_wo(db + 2)
                            w = wot.popleft()
                            po = psum.tile([P, TOK], F32, tag="ps", name="po",
                                           bufs=3)
                            for hp in range(NB):
                                nc.tensor.matmul(po[:], w[:, hp, :],
                                                 AO[hp][:],
                                                 start=(hp == 0),
                                                 stop=(hp == NB - 1))
                            rin = resid[db][:]
                            if rin.dtype == F32R:
                                rin = _f(rin)
                            nc.vector.scalar_tensor_tensor(
                                out_tiles[db][:], po[:], bo[:, db:db + 1],
                                rin, op0=AL.add, op1=AL.add)
                            ln_head(out_tiles, psum, "pkv", st, db)
                        attention.last_ao = AO
                        return st

                    with (
                        tc.tile_pool(name="x1mm", bufs=1) as x1mm,
                        tc.tile_pool(name="wp", bufs=1) as wpool,
                    ):
                        x1 = [x1mm.tile([P, TOK], BF16, tag=f"x1_{d}",
                                        name=f"x1_{d}") for d in range(NB)]
                        mm_t = x1mm.tile([P, NB, L], BF16, tag="mmall")
                        mm = [mm_t[:, t, :] for t in range(NB)]

                        # ------------- self-attention -------------
                        with (
                            tc.tile_pool(name="sa", bufs=1) as sa_pool,
                            tc.tile_pool(name="sa_ps", bufs=1,
                                         space="PSUM") as sa_psum,
                        ):
                            xo_t = sa_pool.tile([P, NB, TOK], BF16,
                                                tag="xoall")
                            nc.sync.dma_start(out=xo_t[:], in_=d_xo[:, :, :])
                            xo = [xo_t[:, t, :] for t in range(NB)]
                            xf_t = sa_pool.tile([P, NB, L], BF16,
                                                tag="xfall")
                            xf = [xf_t[:, t, :] for t in range(NB)]

                            def early():
                                nc.sync.dma_start(out=xf_t[:],
                                                  in_=d_xf[:, :, :])
                                nc.sync.dma_start(out=mm_t[:],
                                                  in_=d_mem[:, :, :])

                            stash = ([dbg, []]
                                     if dbg in ("q", "k", "pav") else None)
                            st = attention(xo, xf, "sa", True, x1, xo,
                                           sa_pool, sa_psum, early_dmas=early,
                                           dbg_stash=stash, q_all=True)
                            if dbg not in ("x1pre", "q", "k", "pav"):
                                ln_tail(x1, "1", sa_psum, "ps", "pva", st,
                                        r_bufs=3)
                            if dbg in ("x1pre", "x1", "ao", "q", "k"):
                                dsrc = (attention.last_ao if dbg == "ao"
                                        else stash[1] if stash else x1)
                                identb = sa_pool.tile([P, P], BF16,
                                                      tag="identb")
                                nc.scalar.copy(out=identb[:], in_=identf[:])
                                osb = [sa_pool.tile([P, D], F32,
                                                    tag=f"dos{s}",
                                                    name=f"dos{s}")
                                       for s in range(SLOTS)]
                                for db in range(NB):
                                    for s in range(SLOTS):
                                        pd = sa_psum.tile([P, P], BF16,
                                                          tag="ps", bufs=3)
                                        nc.tensor.matmul(
                                            pd[:],
                                            dsrc[db][:, s * P:(s + 1) * P],
                                            identb[:], is_transpose=True)
                                        nc.vector.tensor_copy(
                                            osb[s][:, db * P:(db + 1) * P],
                                            pd[:])
                                for s in range(SLOTS):
                                    nc.sync.dma_start(
                                        out=d_out[s * P:(s + 1) * P, :],
                                        in_=osb[s][:])

                        # ------------- cross-attention -------------
                        if dbg is None:
                            with (
                                tc.tile_pool(name="ca", bufs=1) as ca_pool,
                                tc.tile_pool(name="ca_ps", bufs=1,
                                             space="PSUM") as ca_psum,
                            ):
                                st = attention(x1, mm, "ca", False, x2, x1,
                                               ca_pool, ca_psum)
                                ln_tail(x2, "2", ca_psum, "ps", "pva", st,
                                        r_bufs=3)

                    # ---------------- FFN ----------------
                    if dbg is not None:
                        continue
                    with (
                        tc.tile_pool(name="ff", bufs=1) as ff_pool,
                        tc.tile_pool(name="ff_ps", bufs=1,
                                     space="PSUM") as ff_psum,
                    ):
                        w1q = deque()
                        w2q = deque()

                        def fetch_w2(db):
                            t = ff_pool.tile([P, NF, P], BF16, tag="w2t",
                                             bufs=2)
                            nc.sync.dma_start(out=t[:], in_=d_fc2[db])
                            w2q.append(t)

                        def fetch_w1(ff):
                            t = ff_pool.tile([P, NB, P], BF16, tag="w1t",
                                             bufs=3)
                            nc.sync.dma_start(out=t[:], in_=d_fc1[ff])
                            w1q.append(t)

                        fetch_w1(0)
                        fetch_w1(1)
                        ht = []
                        for ff in range(NF):
                            if ff + 2 < NF:
                                fetch_w1(ff + 2)
                            if ff == 16:
                                fetch_w2(0)
                            if ff == 18:
                                fetch_w2(1)
                            w1 = w1q.popleft()
                            pf = ff_psum.tile([P, TOK], F32, tag="pf",
                                              name="pf", bufs=2)
                            for t in range(NB):
                                nc.tensor.matmul(pf[:], w1[:, t, :],
                                                 x2[t][:],
                                                 start=(t == 0),
                                                 stop=(t == NB - 1))
                            h = ff_pool.tile([P, TOK], BF16, tag=f"ht{ff}",
                                             name=f"ht{ff}")
                            nc.scalar.activation(h[:], pf[:], AF.Relu,
                                                 bias=fc1b[:, ff:ff + 1],
                                                 scale=1.0)
                            ht.append(h)
                        x3 = [ff_pool.tile([P, TOK], F32R, tag=f"x3_{d}",
                                           name=f"x3_{d}")
                              for d in range(NB)]
                        st = {}
                        for db in range(NB):
                            if db + 2 < NB:
                                fetch_w2(db + 2)
                            w2 = w2q.popleft()
                            pf2 = ff_psum.tile([P, TOK], F32, tag="pf2",
                                               name="pf2", bufs=2)
                            for t in range(NF):
                                nc.tensor.matmul(pf2[:], w2[:, t, :],
                                                 ht[t][:],
                                                 start=(t == 0),
                                                 stop=(t == NF - 1))
                            nc.vector.scalar_tensor_tensor(
                                x3[db][:], pf2[:], fc2b[:, db:db + 1],
                                x2[db][:], op0=AL.add, op1=AL.add)
                            ln_head(x3, ff_psum, "pf", st, db, bf=False)

                        outsb = [ff_pool.tile([P, D], F32, tag=f"os{s}",
                                              name=f"os{s}")
                                 for s in range(SLOTS)]

                        def post_blk(db):
                            for s in range(SLOTS):
                                ptr = ff_psum.tile([P, P], F32, tag="ptr",
                                                   name="ptr", bufs=2)
                                nc.tensor.matmul(
                                    _r(ptr[:]),
                                    _r(x3[db][:, s * P:(s + 1) * P]),
                                    _r(ident[:]), is_transpose=True)
                                nc.vector.tensor_copy(
                                    outsb[s][:, db * P:(db + 1) * P],
                                    ptr[:])
                            if db == 3:
                                for s in range(SLOTS):
                                    nc.sync.dma_start(
                                        out=d_out[s * P:(s + 1) * P, 0:512],
                                        in_=outsb[s][:, 0:512])

                        ln_tail(x3, "3", ff_psum, "pf2", "ptr", st,
                                post_blk=post_blk, bf=False)
                        for s in range(SLOTS):
                            nc.sync.dma_start(
                                out=d_out[s * P:(s + 1) * P, 512:1024],
                                in_=outsb[s][:, 512:1024])

    nc.finalize()
    return nc


@functools.lru_cache(maxsize=4)
def _get_nc(reps=1, dbg=None):
    return _build_nc(reps, dbg)


def _rel_bucket_np(v):
    """T5 causal bucket for relative distance v = q - k (>= 0)."""
    n = np.maximum(v, 0)
    max_exact = NUM_BUCKETS // 2
    nf = np.maximum(n.astype(np.float32), 1.0)
    val_large = max_exact + (
        np.log(nf / max_exact) / math.log(MAX_DISTANCE / max_exact)
        * (NUM_BUCKETS - max_exact)
    ).astype(np.int32)
    val_large = np.minimum(val_large, NUM_BUCKETS - 1)
    return np.where(n < max_exact, n, val_large).astype(np.int32)


def _build_eb(rel_emb, g):
    """EB[h, i, w] = exp(band_h((w - 128 + 128 g) - i)); 0 where q < k."""
    v = (np.arange(1024)[None, :] - 128 + 128 * g) - np.arange(P)[:, None]
    bucket = _rel_bucket_np(v)                      # [128, 1024]
    band = rel_emb[bucket]                          # [128, 1024, 16]
    band = np.transpose(band, (2, 0, 1)).astype(np.float64)  # [16, 128, 1024]
    eb = np.exp(band)
    eb[:, v < 0] = 0.0
    out = np.zeros((H, P, 1280), dtype=np.float32)
    out[:, :, :1024] = eb
    return out.astype(ml_dtypes.bfloat16)


def _rearr_bias(b):
    return np.ascontiguousarray(b.reshape(-1, P).T, dtype=np.float32)


def _tile4(w, dt=ml_dtypes.bfloat16):
    kb, mb = w.shape[0] // P, w.shape[1] // P
    return np.ascontiguousarray(
        w.reshape(kb, P, mb, P).transpose(2, 1, 0, 3)).astype(dt)


def _tile_v2(w):
    """[K, M] -> [M//256, K//128, 128(k), 256(m)] for the V^T moving op."""
    kb, mb2 = w.shape[0] // P, w.shape[1] // 256
    r = w.reshape(kb, P, mb2, 256).transpose(2, 1, 0, 3)
    # want [pair, P(k_in), kb, 256] with partition dim = k_in
    return np.ascontiguousarray(r).astype(ml_dtypes.bfloat16)


def _gb_stack(g, b):
    return np.ascontiguousarray(
        np.stack([g.reshape(NB, P), b.reshape(NB, P)], axis=0)
    ).astype(ml_dtypes.bfloat16)


def _make_in_maps(inp):
    x = np.asarray(inp["x"], np.float32)
    mem = np.asarray(inp["mem"], np.float32)
    rel_emb = np.asarray(inp["rel_emb"], np.float32)

    shared = {}
    for k in ("sa_wq", "sa_wk", "sa_wo", "ca_wq", "ca_wk", "ca_wo",
              "fc1_w", "fc2_w"):
        shared[k] = _tile4(np.asarray(inp[k]))
    shared["sa_wv2"] = _tile_v2(np.asarray(inp["sa_wv"]))
    shared["ca_wv2"] = _tile_v2(np.asarray(inp["ca_wv"]))
    cols = []
    for pre in ("sa", "ca"):
        bo = np.asarray(inp[f"{pre}_bo"]) + \
            np.asarray(inp[f"{pre}_bv"]) @ np.asarray(inp[f"{pre}_wo"])
        cols.append(_rearr_bias(np.asarray(inp[f"{pre}_bq"])))
        cols.append(_rearr_bias(bo))
    # reorder: sa_bq, sa_bo, ca_bq, ca_bo
    cols = [cols[0], cols[1], cols[2], cols[3],
            _rearr_bias(np.asarray(inp["fc1_b"])),
            _rearr_bias(np.asarray(inp["fc2_b"])),
            _rearr_bias(np.asarray(inp["ln1_g"])),
            _rearr_bias(np.asarray(inp["ln2_g"])),
            _rearr_bias(np.asarray(inp["ln3_g"]))]
    shared["bias_all"] = np.ascontiguousarray(
        np.concatenate(cols, axis=1), np.float32)
    gbs = []
    for i in ("1", "2", "3"):
        gbs.append(np.asarray(inp[f"ln{i}_g"]).reshape(NB, P))
        gbs.append(np.asarray(inp[f"ln{i}_b"]).reshape(NB, P))
    shared["gb_all"] = np.ascontiguousarray(
        np.stack(gbs, axis=0)[None]).astype(ml_dtypes.bfloat16)
    eb = [_build_eb(rel_emb, g) for g in range(2)]
    # [H, P, 1280] -> [NB hp, P, 2, 1280]
    eb = [np.ascontiguousarray(
        e.reshape(NB, 2, P, 1280).transpose(0, 2, 1, 3)) for e in eb]

    def _blk(a):
        # [D, N] -> [P, NB, N]
        return np.ascontiguousarray(
            a.reshape(NB, P, a.shape[1]).transpose(1, 0, 2))

    in_maps = []
    for c in range(8):
        b, g = c // 2, c % 2
        rows = np.concatenate(
            [x[b, (2 * s + g) * P:(2 * s + g + 1) * P] for s in range(SLOTS)])
        m = dict(shared)
        m["xoT"] = _blk(np.ascontiguousarray(rows.T)).astype(
            ml_dtypes.bfloat16)
        m["xfT"] = _blk(np.ascontiguousarray(x[b].T)).astype(
            ml_dtypes.bfloat16)
        m["memT"] = _blk(np.ascontiguousarray(mem[b].T)).astype(
            ml_dtypes.bfloat16)
        m["eb"] = eb[g]
        in_maps.append(m)
    return in_maps


def kernel(x, mem, tgt_mask, mem_mask,
           sa_wq, sa_bq, sa_wk, sa_bk, sa_wv, sa_bv, sa_wo, sa_bo, rel_emb,
           ca_wq, ca_bq, ca_wk, ca_bk, ca_wv, ca_bv, ca_wo, ca_bo,
           fc1_w, fc1_b, fc2_w, fc2_b,
           ln1_g, ln1_b, ln2_g, ln2_b, ln3_g, ln3_b, _trace=False):
    nc = _get_nc()
    in_maps = _make_in_maps(dict(
        x=x, mem=mem, rel_emb=rel_emb,
        sa_wq=sa_wq, sa_wk=sa_wk, sa_wv=sa_wv, sa_wo=sa_wo,
        sa_bq=sa_bq, sa_bk=sa_bk, sa_bv=sa_bv, sa_bo=sa_bo,
        ca_wq=ca_wq, ca_wk=ca_wk, ca_wv=ca_wv, ca_wo=ca_wo,
        ca_bq=ca_bq, ca_bk=ca_bk, ca_bv=ca_bv, ca_bo=ca_bo,
        fc1_w=fc1_w, fc1_b=fc1_b, fc2_w=fc2_w, fc2_b=fc2_b,
        ln1_g=ln1_g, ln1_b=ln1_b, ln2_g=ln2_g, ln2_b=ln2_b,
        ln3_g=ln3_g, ln3_b=ln3_b))

    res = run_bass_kernel_spmd(nc, in_maps, list(range(8)), trace=_trace)
    out = np.empty((B, L, D), np.float32)
    for c in range(8):
        b, g = c // 2, c % 2
        oc = res.results[c]["out_own"]
        for s in range(SLOTS):
            out[b, (2 * s + g) * P:(2 * s + g + 1) * P] = \
                oc[s * P:(s + 1) * P]
    kernel.last_exec_time_ns = res.exec_time_ns
    return out


# revision 79
# speedup vs baseline: 1179.0981x; 1.0007x over previous
"""Trainium2 Bass kernel for a T5-style decoder layer (self-attn with causal
rel-pos bias, cross-attn, FFN, 3 post-LNs).

Sharding: 8 cores = (batch b in 0..3) x (parity g in 0..1). Core (b, g) owns
query blocks {g, 2+g, 4+g, 6+g} (128 rows each) of batch b and computes the
full layer for those 512 rows. K/V work is duplicated across the pair; no
cross-core collectives are needed. Causal score work is padded to a uniform
(2,4,6,8) key-block pattern so one SPMD program serves all cores; padded
blocks are killed by the exp-band (host bakes exp(bias+mask), 0 where
masked).

v2 design notes (vs v0 baseline):
- all matmul *moving* operands are bf16 (cycles/row = 1.0 at any free size);
  residual stream x1/x2/x3 stays f32r for accuracy.
- rel-pos bias enters as EB = exp(band) bf16, applied by a DVE multiply on
  the exp'd scores (2x DVE mode) instead of an identity-matmul accumulate.
- K/V biases dropped: bk shifts every score of a query equally (softmax
  invariant); bv contributes bv@wo to the output (folded into bo on host).
- V^T is built directly (out[key, vdim] = sum_d x[d,key] * wv[d,vdim]) per
  hp-pair with free=256, killing the per-head PE transposes.
- LN: out = x*(g (x) rsd) + R with rank-2 R = g (x) nmr + b (x) 1 computed by
  one PE matmul per block; mean/var reductions interleaved into the Wo / fc2
  loops; reciprocal_approx_fast everywhere.
- weight/EB/mem prefetch via rotating pools so the PE never waits on DMA at
  phase boundaries.
"""

import functools
import math
from collections import deque

import ml_dtypes
import numpy as np

import concourse.bass as bass
import concourse.bacc as bacc
import concourse.mybir as mybir
import concourse.tile as tile
from concourse.bass_utils import run_bass_kernel_spmd
from concourse.masks import make_identity

F32 = mybir.dt.float32
F32R = mybir.dt.float32r
BF16 = mybir.dt.bfloat16
AL = mybir.AluOpType
AF = mybir.ActivationFunctionType

B, L, D, H, DK, DFF = 4, 1024, 1024, 16, 64, 4096
P = 128
NB = D // P            # 8 d_model blocks
NF = DFF // P          # 32 d_ff blocks
TOK = 512              # tokens owned per core
SLOTS = 4              # query blocks of 128 per core
NPAIR = 4              # hp pairs (each pair = 4 heads = 256 qkv dims)
NUM_BUCKETS, MAX_DISTANCE = 32, 128
EPS = 1e-5


def _r(x):
    return x.bitcast(F32R)


def _f(x):
    return x.bitcast(F32)


def _build_nc(reps=1, dbg=None):
    nc = bacc.Bacc(trn_type="TRN2")

    def inp(name, shape, dt=F32):
        return nc.declare_dram_parameter(name, list(shape), dt, isOutput=False)

    d_xo = inp("xoT", (P, NB, TOK), BF16)   # own tokens: Q moving + residual
    d_xf = inp("xfT", (P, NB, L), BF16)     # all tokens, self K/V source
    d_mem = inp("memT", (P, NB, L), BF16)   # memory, cross K/V source
    d_eb = inp("eb", (NB, P, 2, 1280), BF16)  # exp(band), paired per hp
    d_ball = inp("bias_all", (P, 96))        # packed per-partition consts
    d_gball = inp("gb_all", (1, 6, NB, P), BF16)  # LN g/b rows
    dw = {}
    for pre in ("sa", "ca"):
        for nm in ("wq", "wk", "wo"):
            dw[f"{pre}_{nm}"] = inp(f"{pre}_{nm}", (NB, P, NB, P), BF16)
        dw[f"{pre}_wv2"] = inp(f"{pre}_wv2", (NPAIR, P, NB, 256), BF16)
    d_fc1 = inp("fc1_w", (NF, P, NB, P), BF16)
    d_fc2 = inp("fc2_w", (NB, P, NF, P), BF16)
    d_out = nc.declare_dram_parameter("out_own", [TOK, D], F32, isOutput=True)

    with (
        nc.allow_low_precision(reason="bf16 matmul streams; tol 2e-2"),
        tile.TileContext(nc) as tc,
    ):
        with tc.tile_pool(name="persist", bufs=1) as pers:
            identf = pers.tile([P, P], F32, tag="identf")
            make_identity(nc, identf[:])
            ident = pers.tile([P, P], F32R, tag="ident")
            nc.scalar.copy(out=ident[:], in_=identf[:])
            ones_col = pers.tile([P, 1], F32R, tag="ones")
            nc.gpsimd.memset(_f(ones_col[:]), 1.0)
            ones_bf = pers.tile([P, 1], BF16, tag="onesb")
            nc.gpsimd.memset(ones_bf[:], 1.0)
            ones_row = pers.tile([1, P], F32R, tag="onesr")
            nc.gpsimd.memset(_f(ones_row[:]), 1.0)
            ones_row_bf = pers.tile([1, P], BF16, tag="onesrb")
            nc.gpsimd.memset(ones_row_bf[:], 1.0)
            eps_t = pers.tile([1, 1], F32, tag="epsc")
            nc.gpsimd.memset(eps_t[:], EPS)

            ball = pers.tile([P, 96], F32, tag="ball")
            nc.sync.dma_start(out=ball[:], in_=d_ball[:, :])
            gball = pers.tile([1, 6, NB, P], BF16, tag="gball")
            nc.sync.dma_start(out=gball[:], in_=d_gball[:, :, :, :])
            bias_sb = {
                "sa_bq": ball[:, 0:8], "sa_bo": ball[:, 8:16],
                "ca_bq": ball[:, 16:24], "ca_bo": ball[:, 24:32],
            }
            fc1b = ball[:, 32:64]
            fc2b = ball[:, 64:72]
            ln_sb = {}
            for ii, i in enumerate(("1", "2", "3")):
                ln_sb[f"g{i}"] = ball[:, 72 + 8 * ii:80 + 8 * ii]
                ln_sb[f"gr{i}"] = gball[:, 2 * ii, :, :]
                ln_sb[f"br{i}"] = gball[:, 2 * ii + 1, :, :]

            for _rep in range(reps):
                with tc.tile_pool(name="outer", bufs=1) as outer:
                    x2 = [outer.tile([P, TOK], BF16, tag=f"x2_{d}",
                                     name=f"x2_{d}") for d in range(NB)]
                    nmr_t = outer.tile([1, TOK], BF16, tag="nmr")
                    ones_tok = outer.tile([1, TOK], BF16, tag="onestok")
                    nc.gpsimd.memset(ones_tok[:], 1.0)
                    rsd_t = outer.tile([1, TOK], F32, tag="rsd")
                    rsd_bf = outer.tile([1, TOK], BF16, tag="rsdb")
                    rsdr = outer.tile([P, TOK], BF16, tag="rsdr")

                    # ---- layernorm helpers (head interleaved with caller
                    # loop via ln_head(db); tail emits per-block outputs) ----
                    def ln_head(src, psum, acc_tag, state, db, bf=True):
                        ones = ones_bf if bf else ones_col
                        if db == 0:
                            state["pm"] = psum.tile([1, TOK], F32, tag=acc_tag,
                                                    name="pm", bufs=2)
                            state["pv2"] = psum.tile([1, TOK], F32,
                                                     tag=acc_tag, name="pv2",
                                                     bufs=2)
                        nc.tensor.matmul(state["pm"][:], ones[:],
                                         src[db][:],
                                         start=(db == 0), stop=(db == NB - 1))
                        if bf:
                            sq = outer.tile([P, TOK], BF16, tag="sqb", bufs=2)
                            nc.scalar.square(sq[:], src[db][:])
                        else:
                            sq = outer.tile([P, TOK], F32R, tag="sq", bufs=2)
                            nc.scalar.square(sq[:], _f(src[db][:]))
                        nc.tensor.matmul(state["pv2"][:], ones[:],
                                         sq[:],
                                         start=(db == 0), stop=(db == NB - 1))

                    def ln_tail(src, gi, psum, r_tag, bc_tag, state,
                                post_blk=None, r_bufs=2, bf=True):
                        g_ap = ln_sb[f"g{gi}"]
                        gr_ap = ln_sb[f"gr{gi}"]
                        br_ap = ln_sb[f"br{gi}"]
                        pm, pv2 = state["pm"], state["pv2"]
                        mu = outer.tile([1, TOK], F32, tag="mu")
                        nc.vector.tensor_scalar_mul(mu[:], pm[:], 1.0 / D)
                        musq = outer.tile([1, TOK], F32, tag="musq")
                        nc.vector.tensor_mul(musq[:], mu[:], mu[:])
                        var = outer.tile([1, TOK], F32, tag="var")
                        nc.vector.scalar_tensor_tensor(
                            var[:], pv2[:], 1.0 / D, musq[:],
                            op0=AL.mult, op1=AL.subtract)
                        std = outer.tile([1, TOK], F32, tag="std")
                        nc.scalar.activation(std[:], var[:], AF.Sqrt,
                                             bias=eps_t[:])
                        std_c = outer.tile([1, TOK], F32, tag="stdc")
                        nc.vector.tensor_copy(std_c[:], std[:])
                        nc.vector.reciprocal_approx_fast(rsd_t[:], std_c[:])
                        nc.vector.tensor_copy(rsd_bf[:], rsd_t[:])
                        nc.vector.scalar_tensor_tensor(
                            nmr_t[:], mu[:], -1.0, rsd_t[:],
                            op0=AL.mult, op1=AL.mult)
                        pbc = psum.tile([P, TOK], F32, tag=bc_tag, name="pbc",
                                        bufs=r_bufs)
                        nc.tensor.matmul(pbc[:], ones_row_bf[:],
                                         rsd_bf[:])
                        nc.scalar.copy(out=rsdr[:], in_=pbc[:])
                        for db in range(NB):
                            pr = psum.tile([P, TOK], F32, tag=r_tag, name="pr",
                                           bufs=r_bufs)
                            nc.tensor.matmul(pr[:], gr_ap[:, db, :],
                                             nmr_t[:],
                                             start=True, stop=False)
                            nc.tensor.matmul(pr[:], br_ap[:, db, :],
                                             ones_tok[:],
                                             start=False, stop=True)
                            if bf:
                                t1 = outer.tile([P, TOK], BF16, tag="t1b",
                                                bufs=2)
                                nc.vector.scalar_tensor_tensor(
                                    t1[:], src[db][:], g_ap[:, db:db + 1],
                                    rsdr[:], op0=AL.mult, op1=AL.mult)
                            else:
                                t1 = outer.tile([P, TOK], F32, tag="t1",
                                                bufs=2)
                                nc.vector.scalar_tensor_tensor(
                                    t1[:], _f(src[db][:]),
                                    g_ap[:, db:db + 1],
                                    rsdr[:], op0=AL.mult, op1=AL.mult)
                            nc.vector.tensor_add(src[db][:], t1[:], pr[:])
                            if post_blk is not None:
                                post_blk(db)

                    # ================= attention =================
                    def attention(q_src, kvT, pre, causal, out_tiles, resid,
                                  pool, psum, early_dmas=None, dbg_stash=None,
                                  q_all=False):
                        wqv, wkv = dw[f"{pre}_wq"], dw[f"{pre}_wk"]
                        wvv, wov = dw[f"{pre}_wv2"], dw[f"{pre}_wo"]
                        bq, bo = bias_sb[f"{pre}_bq"], bias_sb[f"{pre}_bo"]
                        AO = [pool.tile([P, TOK], BF16, tag=f"ao{hp}",
                                        name=f"ao{hp}", bufs=1)
                              for hp in range(NB)]

                        def fetch(p, with_q=True):
                            ws = {}
                            for j in range(2):
                                hp = 2 * p + j
                                if with_q:
                                    wq_t = wpool.tile([P, NB, P], BF16,
                                                      tag="wqt", bufs=8)
                                    nc.sync.dma_start(out=wq_t[:],
                                                      in_=wqv[hp])
                                    ws[f"wq{j}"] = wq_t
                                wk_t = wpool.tile([P, NB, P], BF16, tag="wkt",
                                                  bufs=4)
                                nc.sync.dma_start(out=wk_t[:], in_=wkv[hp])
                                ws[f"wk{j}"] = wk_t
                                if causal:
                                    eb = pool.tile([P, 2, 1280], BF16,
                                                   tag="ebt", bufs=2)
                                    nc.sync.dma_start(out=eb[:],
                                                      in_=d_eb[hp])
                                    ws[f"eb{j}"] = eb
                            wv_t = wpool.tile([P, NB, 256], BF16, tag="wvt",
                                              bufs=2)
                            nc.sync.dma_start(out=wv_t[:], in_=wvv[p])
                            ws["wv"] = wv_t
                            return ws

                        q_all_sb = []
                        if q_all:
                            wq_ts = []
                            for hp in range(NB):
                                wq_t = wpool.tile([P, NB, P], BF16,
                                                  tag="wqt", bufs=8)
                                nc.sync.dma_start(out=wq_t[:], in_=wqv[hp])
                                wq_ts.append(wq_t)
                            if early_dmas is not None:
                                early_dmas()
                            cur = fetch(0, with_q=False)
                            for hp in range(NB):
                                pq = psum.tile([P, TOK], F32, tag="ps",
                                               name="pq", bufs=3)
                                for t in range(NB):
                                    nc.tensor.matmul(
                                        pq[:], wq_ts[hp][:, t, :],
                                        q_src[t][:],
                                        start=(t == 0), stop=(t == NB - 1))
                                qs = pool.tile([P, TOK], BF16, tag="qsb",
                                               name="qsf", bufs=8)
                                nc.vector.tensor_scalar_add(
                                    qs[:], pq[:], bq[:, hp:hp + 1])
                                q_all_sb.append(qs)
                        else:
                            cur = fetch(0)
                            if early_dmas is not None:
                                early_dmas()

                        wot = deque()

                        def fetch_wo(db):
                            t = wpool.tile([P, NB, P], BF16, tag="wot",
                                           bufs=2)
                            nc.sync.dma_start(out=t[:], in_=wov[db])
                            wot.append(t)

                        pend = []   # deferred normalize from previous pair
                        for p in range(NPAIR):
                            if p == NPAIR - 1:
                                fetch_wo(0)
                                fetch_wo(1)
                            nxt = (fetch(p + 1, with_q=not q_all)
                                   if p + 1 < NPAIR else None)
                            # flush previous pair's normalize (recips done)
                            for fn in pend:
                                fn()
                            pend = []

                            # ---- projections ----
                            q_sb, k_sb, vh = [], [], []

                            def do_q():
                                if q_all:
                                    q_sb.extend(q_all_sb[2 * p:2 * p + 2])
                                    return
                                for j in range(2):
                                    hp = 2 * p + j
                                    pq = psum.tile([P, TOK], F32, tag="ps",
                                                   name="pq", bufs=3)
                                    for t in range(NB):
                                        nc.tensor.matmul(
                                            pq[:], cur[f"wq{j}"][:, t, :],
                                            q_src[t][:],
                                            start=(t == 0),
                                            stop=(t == NB - 1))
                                    qs = pool.tile([P, TOK], BF16, tag="qsb",
                                                   bufs=8)
                                    nc.vector.tensor_scalar_add(
                                        qs[:], pq[:], bq[:, hp:hp + 1])
                                    q_sb.append(qs)
                                    if dbg_stash is not None and \
                                            dbg_stash[0] == "q":
                                        dt_ = pool.tile([P, TOK], BF16,
                                                        tag=f"dq{hp}",
                                                        name=f"dq{hp}")
                                        nc.vector.tensor_copy(dt_[:], qs[:])
                                        dbg_stash[1].append(dt_)

                            def do_kv():
                                for j in range(2):
                                    ks = pool.tile([P, L], BF16, tag="ksb",
                                                   bufs=2)
                                    for half in range(2):
                                        sl = slice(half * 512,
                                                   (half + 1) * 512)
                                        pk = psum.tile([P, 512], F32,
                                                       tag="pkv", name="pk",
                                                       bufs=2)
                                        for t in range(NB):
                                            nc.tensor.matmul(
                                                pk[:], cur[f"wk{j}"][:, t, :],
                                                kvT[t][:, sl],
                                                start=(t == 0),
                                                stop=(t == NB - 1))
                                        nc.vector.tensor_copy(ks[:, sl],
                                                              pk[:])
                                    k_sb.append(ks)
                                    if dbg_stash is not None and \
                                            dbg_stash[0] == "k":
                                        hp = 2 * p + j
                                        dt_ = pool.tile([P, TOK], BF16,
                                                        tag=f"dk{hp}",
                                                        name=f"dk{hp}")
                                        nc.vector.tensor_copy(
                                            dt_[:], ks[:, 0:TOK])
                                        dbg_stash[1].append(dt_)
                                # V^T direct: vh[kb][key, head_in_pair, 0:64]
                                for kb in range(8):
                                    vt = pool.tile([P, 4, 65], BF16,
                                                   tag=f"vh{kb}",
                                                   name=f"vh{kb}", bufs=1)
                                    nc.gpsimd.memset(vt[:], 1.0)
                                    pvt = psum.tile([P, 256], F32, tag="pva",
                                                    name="pvt", bufs=3)
                                    for t in range(NB):
                                        nc.tensor.matmul(
                                            pvt[:],
                                            kvT[t][:, kb * P:(kb + 1) * P],
                                            cur["wv"][:, t, :],
                                            start=(t == 0),
                                            stop=(t == NB - 1))
                                    nc.vector.tensor_copy(
                                        vt[:, :, 0:64],
                                        pvt[:].rearrange("p (h c) -> p h c",
                                                         c=64))
                                    vh.append(vt)

                            if causal:
                                do_q()
                                do_kv()
                            else:
                                do_kv()
                                do_q()
                            if dbg_stash is not None and \
                                    dbg_stash[0] in ("q", "k"):
                                cur = nxt
                                continue

                            # ---- scores / softmax / AV ----
                            pts_hh = {}
                            pav_hh = {}
                            rec_hh = {}

                            def scores(hh):
                                j, lo = hh // 2, (hh % 2) * 64
                                hsl = slice(lo, lo + 64)
                                pts = []
                                for kb in range(8):
                                    smin = kb // 2 if causal else 0
                                    n = TOK - smin * P
                                    ns = SLOTS - smin
                                    ps = psum.tile([P, TOK], F32, tag="ps",
                                                   name="ps", bufs=3)
                                    nc.tensor.matmul(
                                        ps[:, 0:n],
                                        k_sb[j][hsl, kb * P:(kb + 1) * P],
                                        q_sb[j][hsl, smin * P:TOK])
                                    if causal:
                                        pe = pool.tile([P, TOK], BF16,
                                                       tag="pe", bufs=2)
                                        nc.scalar.activation(
                                            pe[:, 0:n], ps[:, 0:n], AF.Exp,
                                            scale=0.125)
                                        w0 = 256 * smin - 128 * kb + 128
                                        ebv = cur[f"eb{j}"][
                                            :, hh % 2,
                                            w0:w0 + ns * 256].rearrange(
                                            "p (s c) -> p s c",
                                            c=256)[:, :, 0:P]
                                        pt = pool.tile([P, TOK], BF16,
                                                       tag="pt", bufs=16)
                                        nc.vector.tensor_mul(
                                            pt[:, 0:n].rearrange(
                                                "p (s c) -> p s c", c=P),
                                            pe[:, 0:n].rearrange(
                                                "p (s c) -> p s c", c=P),
                                            ebv)
                                    else:
                                        pt = pool.tile([P, TOK], BF16,
                                                       tag="pe", bufs=16)
                                        nc.scalar.activation(
                                            pt[:, 0:n], ps[:, 0:n], AF.Exp,
                                            scale=0.125)
                                    pts.append(pt)
                                pts_hh[hh] = pts

                            def pav_f(hh):
                                pav = psum.tile([65, TOK], F32, tag="pva",
                                                name="pav", bufs=3)
                                for kb in range(8):
                                    smin = kb // 2 if causal else 0
                                    n = TOK - smin * P
                                    nc.tensor.matmul(
                                        pav[:, smin * P:TOK],
                                        vh[kb][:, hh, :],
                                        pts_hh[hh][kb][:, 0:n],
                                        start=(kb == 0), stop=(kb == 7))
                                # copy den to SBUF on DVE first: the custom
                                # DVE recip lacks cross-engine dep tracking,
                                # in-order DVE queue makes this safe
                                dsb = pool.tile([1, TOK], F32, tag="dens",
                                                bufs=2)
                                nc.vector.tensor_copy(dsb[:], pav[64:65, :])
                                rec = pool.tile([1, TOK], F32, tag="rec",
                                                bufs=2)
                                nc.vector.reciprocal_approx_fast(
                                    rec[:], dsb[:])
                                rcb = pool.tile([1, TOK], BF16, tag="recb",
                                                bufs=2)
                                nc.vector.tensor_copy(rcb[:], rec[:])
                                pav_hh[hh] = pav
                                rec_hh[hh] = rcb
                                if dbg_stash is not None and \
                                        dbg_stash[0] == "pav" and \
                                        p == 0 and hh == 0:
                                    dnum = pool.tile([64, TOK], F32,
                                                     tag="dnum")
                                    nc.vector.tensor_copy(dnum[:],
                                                          pav[0:64, :])
                                    dden = pool.tile([1, TOK], F32,
                                                     tag="dden")
                                    nc.vector.tensor_copy(dden[:],
                                                          pav[64:65, :])
                                    nc.sync.dma_start(
                                        out=d_out[0:64, 0:TOK],
                                        in_=dnum[:])
                                    nc.sync.dma_start(
                                        out=d_out[64:65, 0:TOK],
                                        in_=dden[:])
                                    nc.sync.dma_start(
                                        out=d_out[65:66, 0:TOK],
                                        in_=rec[:])

                            def norm_f(hh, hp, hsl):
                                def run():
                                    prr = psum.tile([64, TOK], F32, tag="ps",
                                                    name="prr", bufs=3)
                                    nc.tensor.matmul(
                                        prr[:], ones_row_bf[0:1, 0:64],
                                        rec_hh[hh][:])
                                    rrep = pool.tile([64, TOK], F32,
                                                     tag="rrep", bufs=2)
                                    nc.scalar.copy(out=rrep[:], in_=prr[:])
                                    nc.vector.tensor_mul(
                                        AO[hp][hsl, :], pav_hh[hh][0:64, :],
                                        rrep[:])
                                return run

                            scores(0)
                            scores(1)
                            pav_f(0)
                            scores(2)
                            pav_f(1)
                            norm_f(0, 2 * p, slice(0, 64))()
                            scores(3)
                            pav_f(2)
                            norm_f(1, 2 * p, slice(64, 128))()
                            pav_f(3)
                            pend = [norm_f(2, 2 * p + 1, slice(0, 64)),
                                    norm_f(3, 2 * p + 1, slice(64, 128))]
                            cur = nxt
                        if dbg_stash is not None:
                            for fn in pend:
                                fn()
                            attention.last_ao = AO
                            return {}

                        # ---- Wo + residual (+ LN head interleaved) ----
                        # last pair's deferred normalize (DVE recips) flushes
                        # between po(db0)'s hp<=6 and hp==7 accumulation so
                        # the PE keeps streaming while DVE catches up
                        st = {}
                        for db in range(NB):
                            if db + 2 < NB:
                                fetch_wo(db + 2)
                            w = wot.popleft()
                            po = psum.tile([P, TOK], F32, tag="ps", name="po",
                                           bufs=3)
                            for hp in range(NB - 1):
                                nc.tensor.matmul(po[:], w[:, hp, :],
                                                 AO[hp][:],
                                                 start=(hp == 0),
                                                 stop=False)
                            if db == 0:
                                for fn in pend:
                                    fn()
                                pend = []
                            nc.tensor.matmul(po[:], w[:, NB - 1, :],
                                             AO[NB - 1][:],
                                             start=False, stop=True)
                            rin = resid[db][:]
                            if rin.dtype == F32R:
                                rin = _f(rin)
                            nc.vector.scalar_tensor_tensor(
                                out_tiles[db][:], po[:], bo[:, db:db + 1],
                                rin, op0=AL.add, op1=AL.add)
                            ln_head(out_tiles, psum, "pkv", st, db)
                        attention.last_ao = AO
                        return st

                    with (
                        tc.tile_pool(name="x1mm", bufs=1) as x1mm,
                        tc.tile_pool(name="wp", bufs=1) as wpool,
                    ):
                        x1 = [x1mm.tile([P, TOK], BF16, tag=f"x1_{d}",
                                        name=f"x1_{d}") for d in range(NB)]
                        mm_t = x1mm.tile([P, NB, L], BF16, tag="mmall")
                        mm = [mm_t[:, t, :] for t in range(NB)]

                        # ------------- self-attention -------------
                        with (
                            tc.tile_pool(name="sa", bufs=1) as sa_pool,
                            tc.tile_pool(name="sa_ps", bufs=1,
                                         space="PSUM") as sa_psum,
                        ):
                            xo_t = sa_pool.tile([P, NB, TOK], BF16,
                                                tag="xoall")
                            nc.sync.dma_start(out=xo_t[:], in_=d_xo[:, :, :])
                            xo = [xo_t[:, t, :] for t in range(NB)]
                            xf_t = sa_pool.tile([P, NB, L], BF16,
                                                tag="xfall")
                            xf = [xf_t[:, t, :] for t in range(NB)]

                            def early():
                                nc.sync.dma_start(out=xf_t[:],
                                                  in_=d_xf[:, :, :])
                                nc.sync.dma_start(out=mm_t[:],
                                                  in_=d_mem[:, :, :])

                            stash = ([dbg, []]
                                     if dbg in ("q", "k", "pav") else None)
                            st = attention(xo, xf, "sa", True, x1, xo,
                                           sa_pool, sa_psum, early_dmas=early,
                                           dbg_stash=stash, q_all=True)
                            if dbg not in ("x1pre", "q", "k", "pav"):
                                ln_tail(x1, "1", sa_psum, "ps", "pva", st,
                                        r_bufs=3)
                            if dbg in ("x1pre", "x1", "ao", "q", "k"):
                                dsrc = (attention.last_ao if dbg == "ao"
                                        else stash[1] if stash else x1)
                                identb = sa_pool.tile([P, P], BF16,
                                                      tag="identb")
                                nc.scalar.copy(out=identb[:], in_=identf[:])
                                osb = [sa_pool.tile([P, D], F32,
                                                    tag=f"dos{s}",
                                                    name=f"dos{s}")
                                       for s in range(SLOTS)]
                                for db in range(NB):
                                    for s in range(SLOTS):
                                        pd = sa_psum.tile([P, P], BF16,
                                                          tag="ps", bufs=3)
                                        nc.tensor.matmul(
                                            pd[:],
                                            dsrc[db][:, s * P:(s + 1) * P],
                                            identb[:], is_transpose=True)
                                        nc.vector.tensor_copy(
                                            osb[s][:, db * P:(db + 1) * P],
                                            pd[:])
                                for s in range(SLOTS):
                                    nc.sync.dma_start(
                                        out=d_out[s * P:(s + 1) * P, :],
                                        in_=osb[s][:])

                        # ------------- cross-attention -------------
                        if dbg is None:
                            with (
                                tc.tile_pool(name="ca", bufs=1) as ca_pool,
                                tc.tile_pool(name="ca_ps", bufs=1,
                                             space="PSUM") as ca_psum,
                            ):
                                st = attention(x1, mm, "ca", False, x2, x1,
                                               ca_pool, ca_psum)
                                ln_tail(x2, "2", ca_psum, "ps", "pva", st,
                                        r_bufs=3)

                    # ---------------- FFN ----------------
                    if dbg is not None:
                        continue
                    with (
                        tc.tile_pool(name="ff", bufs=1) as ff_pool,
                        tc.tile_pool(name="ff_ps", bufs=1,
                                     space="PSUM") as ff_psum,
                    ):
                        w1q = deque()
                        w2q = deque()

                        def fetch_w2(db):
                            t = ff_pool.tile([P, NF, P], BF16, tag="w2t",
                                             bufs=2)
                            nc.sync.dma_start(out=t[:], in_=d_fc2[db])
                            w2q.append(t)

                        def fetch_w1(ff):
                            t = ff_pool.tile([P, NB, P], BF16, tag="w1t",
                                             bufs=3)
                            nc.sync.dma_start(out=t[:], in_=d_fc1[ff])
                            w1q.append(t)

                        fetch_w1(0)
                        fetch_w1(1)
                        ht = []
                        for ff in range(NF):
                            if ff + 2 < NF:
                                fetch_w1(ff + 2)
                            if ff == 16:
                                fetch_w2(0)
                            if ff == 18:
                                fetch_w2(1)
                            w1 = w1q.popleft()
                            pf = ff_psum.tile([P, TOK], F32, tag="pf",
                                              name="pf", bufs=2)
                            for t in range(NB):
                                nc.tensor.matmul(pf[:], w1[:, t, :],
                                                 x2[t][:],
                                                 start=(t == 0),
                                                 stop=(t == NB - 1))
                            h = ff_pool.tile([P, TOK], BF16, tag=f"ht{ff}",
                                             name=f"ht{ff}")
                            nc.scalar.activation(h[:], pf[:], AF.Relu,
                                                 bias=fc1b[:, ff:ff + 1],
                                                 scale=1.0)
                            ht.append(h)
                        x3 = [ff_pool.tile([P, TOK], F32R, tag=f"x3_{d}",
                                           name=f"x3_{d}")
                              for d in range(NB)]
                        st = {}
                        for db in range(NB):
                            if db + 2 < NB:
                                fetch_w2(db + 2)
                            w2 = w2q.popleft()
                            pf2 = ff_psum.tile([P, TOK], F32, tag="pf2",
                                               name="pf2", bufs=2)
                            for t in range(NF):
                                nc.tensor.matmul(pf2[:], w2[:, t, :],
                                                 ht[t][:],
                                                 start=(t == 0),
                                                 stop=(t == NF - 1))
                            nc.vector.scalar_tensor_tensor(
                                x3[db][:], pf2[:], fc2b[:, db:db + 1],
                                x2[db][:], op0=AL.add, op1=AL.add)
                            ln_head(x3, ff_psum, "pf", st, db, bf=False)

                        outsb = [ff_pool.tile([P, D], F32, tag=f"os{s}",
                                              name=f"os{s}")
                                 for s in range(SLOTS)]

                        def post_blk(db):
                            for s in range(SLOTS):
                                ptr = ff_psum.tile([P, P], F32, tag="ptr",
                                                   name="ptr", bufs=2)
                                nc.tensor.matmul(
                                    _r(ptr[:]),
                                    _r(x3[db][:, s * P:(s + 1) * P]),
                                    _r(ident[:]), is_transpose=True)
                                nc.vector.tensor_copy(
                                    outsb[s][:, db * P:(db + 1) * P],
                                    ptr[:])
                            if db == 3:
                                for s in range(SLOTS):
                                    nc.sync.dma_start(
                                        out=d_out[s * P:(s + 1) * P, 0:512],
                                        in_=outsb[s][:, 0:512])

                        ln_tail(x3, "3", ff_psum, "pf2", "ptr", st,
                                post_blk=post_blk, bf=False)
                        for s in range(SLOTS):
                            nc.sync.dma_start(
                                out=d_out[s * P:(s + 1) * P, 512:1024],
                                in_=outsb[s][:, 512:1024])

    nc.finalize()
    return nc


@functools.lru_cache(maxsize=4)
def _get_nc(reps=1, dbg=None):
    return _build_nc(reps, dbg)


def _rel_bucket_np(v):
    """T5 causal bucket for relative distance v = q - k (>= 0)."""
    n = np.maximum(v, 0)
    max_exact = NUM_BUCKETS // 2
    nf = np.maximum(n.astype(np.float32), 1.0)
    val_large = max_exact + (
        np.log(nf / max_exact) / math.log(MAX_DISTANCE / max_exact)
        * (NUM_BUCKETS - max_exact)
    ).astype(np.int32)
    val_large = np.minimum(val_large, NUM_BUCKETS - 1)
    return np.where(n < max_exact, n, val_large).astype(np.int32)


def _build_eb(rel_emb, g):
    """EB[h, i, w] = exp(band_h((w - 128 + 128 g) - i)); 0 where q < k."""
    v = (np.arange(1024)[None, :] - 128 + 128 * g) - np.arange(P)[:, None]
    bucket = _rel_bucket_np(v)                      # [128, 1024]
    band = rel_emb[bucket]                          # [128, 1024, 16]
    band = np.transpose(band, (2, 0, 1)).astype(np.float64)  # [16, 128, 1024]
    eb = np.exp(band)
    eb[:, v < 0] = 0.0
    out = np.zeros((H, P, 1280), dtype=np.float32)
    out[:, :, :1024] = eb
    return out.astype(ml_dtypes.bfloat16)


def _rearr_bias(b):
    return np.ascontiguousarray(b.reshape(-1, P).T, dtype=np.float32)


def _tile4(w, dt=ml_dtypes.bfloat16):
    kb, mb = w.shape[0] // P, w.shape[1] // P
    return np.ascontiguousarray(
        w.reshape(kb, P, mb, P).transpose(2, 1, 0, 3)).astype(dt)


def _tile_v2(w):
    """[K, M] -> [M//256, K//128, 128(k), 256(m)] for the V^T moving op."""
    kb, mb2 = w.shape[0] // P, w.shape[1] // 256
    r = w.reshape(kb, P, mb2, 256).transpose(2, 1, 0, 3)
    # want [pair, P(k_in), kb, 256] with partition dim = k_in
    return np.ascontiguousarray(r).astype(ml_dtypes.bfloat16)


def _gb_stack(g, b):
    return np.ascontiguousarray(
        np.stack([g.reshape(NB, P), b.reshape(NB, P)], axis=0)
    ).astype(ml_dtypes.bfloat16)


def _make_in_maps(inp):
    x = np.asarray(inp["x"], np.float32)
    mem = np.asarray(inp["mem"], np.float32)
    rel_emb = np.asarray(inp["rel_emb"], np.float32)

    shared = {}
    for k in ("sa_wq", "sa_wk", "sa_wo", "ca_wq", "ca_wk", "ca_wo",
              "fc1_w", "fc2_w"):
        shared[k] = _tile4(np.asarray(inp[k]))
    shared["sa_wv2"] = _tile_v2(np.asarray(inp["sa_wv"]))
    shared["ca_wv2"] = _tile_v2(np.asarray(inp["ca_wv"]))
    cols = []
    for pre in ("sa", "ca"):
        bo = np.asarray(inp[f"{pre}_bo"]) + \
            np.asarray(inp[f"{pre}_bv"]) @ np.asarray(inp[f"{pre}_wo"])
        cols.append(_rearr_bias(np.asarray(inp[f"{pre}_bq"])))
        cols.append(_rearr_bias(bo))
    # reorder: sa_bq, sa_bo, ca_bq, ca_bo
    cols = [cols[0], cols[1], cols[2], cols[3],
            _rearr_bias(np.asarray(inp["fc1_b"])),
            _rearr_bias(np.asarray(inp["fc2_b"])),
            _rearr_bias(np.asarray(inp["ln1_g"])),
            _rearr_bias(np.asarray(inp["ln2_g"])),
            _rearr_bias(np.asarray(inp["ln3_g"]))]
    shared["bias_all"] = np.ascontiguousarray(
        np.concatenate(cols, axis=1), np.float32)
    gbs = []
    for i in ("1", "2", "3"):
        gbs.append(np.asarray(inp[f"ln{i}_g"]).reshape(NB, P))
        gbs.append(np.asarray(inp[f"ln{i}_b"]).reshape(NB, P))
    shared["gb_all"] = np.ascontiguousarray(
        np.stack(gbs, axis=0)[None]).astype(ml_dtypes.bfloat16)
    eb = [_build_eb(rel_emb, g) for g in range(2)]
    # [H, P, 1280] -> [NB hp, P, 2, 1280]
    eb = [np.ascontiguousarray(
        e.reshape(NB, 2, P, 1280).transpose(0, 2, 1, 3)) for e in eb]

    def _blk(a):
        # [D, N] -> [P, NB, N]
        return np.ascontiguousarray(
            a.reshape(NB, P, a.shape[1]).transpose(1, 0, 2))

    in_maps = []
    for c in range(8):
        b, g = c // 2, c % 2
        rows = np.concatenate(
            [x[b, (2 * s + g) * P:(2 * s + g + 1) * P] for s in range(SLOTS)])
        m = dict(shared)
        m["xoT"] = _blk(np.ascontiguousarray(rows.T)).astype(
            ml_dtypes.bfloat16)
        m["xfT"] = _blk(np.ascontiguousarray(x[b].T)).astype(
            ml_dtypes.bfloat16)
        m["memT"] = _blk(np.ascontiguousarray(mem[b].T)).astype(
            ml_dtypes.bfloat16)
        m["eb"] = eb[g]
        in_maps.append(m)
    return in_maps


def kernel(x, mem, tgt_mask, mem_mask,
           sa_wq, sa_bq, sa_wk, sa_bk, sa_wv, sa_bv, sa_wo, sa_bo, rel_emb,
           ca_wq, ca_bq, ca_wk, ca_bk, ca_wv, ca_bv, ca_wo, ca_bo,
           fc1_w, fc1_b, fc2_w, fc2_b,
           ln1_g, ln1_b, ln2_g, ln2_b, ln3_g, ln3_b, _trace=False):
    nc = _get_nc()
    in_maps = _make_in_maps(dict(
        x=x, mem=mem, rel_emb=rel_emb,
        sa_wq=sa_wq, sa_wk=sa_wk, sa_wv=sa_wv, sa_wo=sa_wo,
        sa_bq=sa_bq, sa_bk=sa_bk, sa_bv=sa_bv, sa_bo=sa_bo,
        ca_wq=ca_wq, ca_wk=ca_wk, ca_wv=ca_wv, ca_wo=ca_wo,
        ca_bq=ca_bq, ca_bk=ca_bk, ca_bv=ca_bv, ca_bo=ca_bo,
        fc1_w=fc1_w, fc1_b=fc1_b, fc2_w=fc2_w, fc2_b=fc2_b,
        ln1_g=ln1_g, ln1_b=ln1_b, ln2_g=ln2_g, ln2_b=ln2_b,
        ln3_g=ln3_g, ln3_b=ln3_b))

    res = run_bass_kernel_spmd(nc, in_maps, list(range(8)), trace=_trace)
    out = np.empty((B, L, D), np.float32)
    for c in range(8):
        b, g = c // 2, c % 2
        oc = res.results[c]["out_own"]
        for s in range(SLOTS):
            out[b, (2 * s + g) * P:(2 * s + g + 1) * P] = \
                oc[s * P:(s + 1) * P]
    kernel.last_exec_time_ns = res.exec_time_ns
    return out
